# revision 1
# baseline (speedup 1.0000x reference)
"""AttentionLSTM Trainium2 kernel — 8-core data-parallel.

Model (per batch row b): two independent single-direction LSTMs over T=43
steps of x[:, :, t] (H=300 features), hidden states summed, then a
conv-softmax attention over time, tanh, fc(300->80), softmax.

Device mapping per core (512 batch rows):
  - z^T[1200, 512] per (direction, step) via PE matmuls with K padded
    300->384 (3 k-tiles of 128), M gate-aligned tiles {128,128,44}.
  - MM inputs in 16-bit (fp16 default) at 1 cycle/row; accumulation fp32.
  - gates: ScalarE sigmoid/tanh with fused per-partition bias, VectorE
    fused [sig_i|sig_f] * [tanh_g|c] products, c/h state in SBUF.
  - attention accumulated online: e_t = sigmoid(a)/(1-sigmoid(a)) = exp(a)
    (avoids exp table loads mid-loop); r += hsum_t * e_t on GPSIMD.
  - tail: hStar = tanh(r/s), logits = fc(hStar) via PE (batch on PSUM
    partitions), softmax over the 80-class free dim.
"""

import os
import sys

sys.path.insert(0, "/opt/trn_rl_repo")

from contextlib import ExitStack

import numpy as np

import concourse.bass as bass
import concourse.tile as tile
from concourse import mybir
from concourse.bass_utils import run_bass_kernel_spmd

f32 = mybir.dt.float32
AF = mybir.ActivationFunctionType
AX = mybir.AxisListType

_BIRFIX_DONE = False
DEBUG = False


def _split_multiwaits(bir_json):
    """This walrus build allows one sync-wait per engine instruction; Tile
    attaches one per producer proc. Hoist extras onto standalone
    EventSemaphore instructions inserted just before, same engine queue."""
    import json
    j = json.loads(bir_json.decode() if isinstance(bir_json, bytes) else bir_json)
    for fn in j.get("functions", []):
        for blk in fn.get("blocks", []):
            out = []
            for ins in blk.get("instructions", []):
                si = ins.get("sync_info")
                ow = si.get("on_wait") if si else None
                if ow and len(ow) > 1:
                    for i, w in enumerate(ow[:-1]):
                        out.append({
                            "debug": ins.get("debug", 0),
                            "engine": ins["engine"],
                            "ins": [], "outs": [],
                            "name": f"{ins['name']}_xw{i}",
                            "opcode": "EventSemaphore",
                            "sync_info": {"on_update": [], "on_wait": [w]},
                        })
                    si["on_wait"] = [ow[-1]]
                out.append(ins)
            blk["instructions"] = out
    return json.dumps(j).encode()


def _install_birfix():
    global _BIRFIX_DONE
    if _BIRFIX_DONE:
        return
    from concourse import bass2jax
    orig = bass2jax.compile_bir_kernel

    def patched(bir_json, tmpdir, neff_name="file.neff"):
        return orig(_split_multiwaits(bir_json), tmpdir, neff_name)

    bass2jax.compile_bir_kernel = patched
    _BIRFIX_DONE = True


class _Runner:
    """Compile once; keep the sharded jitted executable + device inputs."""

    def __init__(self, nc, n_cores):
        import jax
        from jax.sharding import Mesh, PartitionSpec
        from jax.experimental.shard_map import shard_map
        from concourse import bass2jax as b2j

        b2j.install_neuronx_cc_hook()
        _install_birfix()
        self.jax = jax
        self.nc = nc
        self.n_cores = n_cores
        part_name = nc.partition_id_tensor.name if nc.partition_id_tensor else None
        in_names, out_names, out_avals, zero_outs = [], [], [], []
        for alloc in nc.m.functions[0].allocations:
            if not isinstance(alloc, mybir.MemoryLocationSet):
                continue
            name = alloc.memorylocations[0].name
            if alloc.kind == "ExternalInput":
                if name != part_name:
                    in_names.append(name)
            elif alloc.kind == "ExternalOutput":
                out_names.append(name)
                shape = tuple(alloc.tensor_shape)
                dtype = mybir.dt.np(alloc.dtype)
                out_avals.append(jax.core.ShapedArray(shape, dtype))
                zero_outs.append(np.zeros(shape, dtype))
        self.in_names = list(in_names)
        self.out_names = out_names
        self.out_avals = out_avals
        self.zero_outs = zero_outs
        n_params = len(in_names)
        n_outs = len(out_avals)
        all_names = in_names + out_names
        if part_name is not None:
            all_names = all_names + [part_name]
        donate = tuple(range(n_params, n_params + n_outs))

        def _body(*args):
            operands = list(args)
            if part_name is not None:
                operands.append(b2j.partition_id_tensor())
            outs = b2j._bass_exec_p.bind(
                *operands,
                out_avals=tuple(out_avals),
                in_names=tuple(all_names),
                out_names=tuple(out_names),
                lowering_input_output_aliases=(),
                sim_require_finite=True,
                sim_require_nnan=True,
                nc=nc,
            )
            return tuple(outs)

        devices = jax.devices()[:n_cores]
        self.mesh = Mesh(np.asarray(devices), ("core",))
        in_specs = (PartitionSpec("core"),) * (n_params + n_outs)
        out_specs = (PartitionSpec("core"),) * n_outs
        self.sharded = jax.jit(
            shard_map(_body, mesh=self.mesh, in_specs=in_specs,
                      out_specs=out_specs, check_rep=False),
            donate_argnums=donate, keep_unused=True)
        self.sharding = jax.sharding.NamedSharding(
            self.mesh, PartitionSpec("core"))

    def put_inputs(self, in_maps):
        jax = self.jax
        concat = [np.concatenate([np.asarray(m[n]) for m in in_maps], axis=0)
                  for n in self.in_names]
        return [jax.device_put(a, self.sharding) for a in concat]

    def call(self, dev_in):
        zeros = [np.zeros((self.n_cores * z.shape[0], *z.shape[1:]), z.dtype)
                 for z in self.zero_outs]
        outs = self.sharded(*dev_in, *zeros)
        self.jax.block_until_ready(outs)
        return outs

    def run(self, in_maps):
        dev_in = self.put_inputs(in_maps)
        outs = self.call(dev_in)
        n = self.n_cores
        return [
            {name: np.asarray(outs[i]).reshape(n, *self.out_avals[i].shape)[c]
             for i, name in enumerate(self.out_names)}
            for c in range(n)
        ]

    def bench(self, in_maps, iters=5):
        import time
        dev_in = self.put_inputs(in_maps)
        self.call(dev_in)  # warm
        times = []
        for _ in range(iters):
            t0 = time.perf_counter()
            self.call(dev_in)
            times.append(time.perf_counter() - t0)
        return times

B, H, T, NCLS = 4096, 300, 43, 80
NCORES = 8
BS = B // NCORES          # 512 batch rows per core
NK = 5                    # merged contraction: x(300)+pad(20)+h(300)+bias(1)+pad
XOFF = 0                  # x rows 0..299
HOFF = 320                # h rows 320..619 (h row r -> ktile (320+r)//128)
BROW = 620                # bias row (constant-1 rhs row, bias vector in weights)
KP = NK * 128             # 640
# gate-row tiles (moff, msz, base): base = partition offset inside the k-tile
# j0: h rows 0..63   -> ktile2 parts 64..127
# j1: h rows 64..191 -> ktile3 parts 0..127
# j2: h rows 192..299-> ktile4 parts 0..107
MT = [(0, 64, 64), (64, 128, 0), (192, 108, 0)]
GATES = [("i", 0), ("f", 300), ("g", 600), ("o", 900)]  # torch order i,f,g,o

MM_DT_NAME = os.environ.get("LSTM_MM_DT", "float16")
TRACE = False
LAST_EXEC_NS = None

_CACHE = {}


def _build(mdt_name, repeat=0):
    mdt = getattr(mybir.dt, mdt_name)
    nc = bass.Bass(target_bir_lowering=False)

    xt_d = nc.declare_dram_parameter("xt", [T, 3, 128, BS], mdt, isOutput=False)
    wc_d = nc.declare_dram_parameter("wc", [2, NK, 128, 1200], mdt, isOutput=False)
    conv_d = nc.declare_dram_parameter("convp", [128, 3], mdt, isOutput=False)
    fcw_d = nc.declare_dram_parameter("fcw", [128, 3 * NCLS], mdt, isOutput=False)
    fcb_d = nc.declare_dram_parameter("fcb", [1, NCLS], mdt, isOutput=False)
    ones_d = nc.declare_dram_parameter("onesrow", [1, BS], mdt, isOutput=False)
    out_d = nc.declare_dram_parameter("out", [BS, NCLS], f32, isOutput=True)
    if DEBUG:
        dbg_a = nc.declare_dram_parameter("dbg_a", [T, BS], f32, isOutput=True)
        dbg_e = nc.declare_dram_parameter("dbg_e", [T, BS], f32, isOutput=True)
        dbg_r = nc.declare_dram_parameter("dbg_r", [3, 128, BS], f32, isOutput=True)
        dbg_s = nc.declare_dram_parameter("dbg_s", [1, BS], f32, isOutput=True)
        dbg_hn = nc.declare_dram_parameter("dbg_hn", [3, 128, BS], f32, isOutput=True)
        dbg_lg = nc.declare_dram_parameter("dbg_lg", [4, 128, NCLS], f32, isOutput=True)

    with tile.TileContext(nc) as tc, ExitStack() as ctx:
        P = lambda name, bufs, **kw: ctx.enter_context(
            tc.tile_pool(name=name, bufs=bufs, **kw))
        wpool = P("w", 1)
        xpool = P("x", 2)
        zpool = P("z", 6, space="PSUM")
        apool = P("aps", 1, space="PSUM")
        ebpp = P("ebps", 1, space="PSUM")
        sifp = P("sif", 3 if mdt_name == "float32" else 4)
        sop = P("so", 4)
        gcp = P("gc", 1)
        p1p = P("p1", 3)
        tcp = P("tc", 3)
        hp = P("h", 1)
        hsp = P("hs", 2)
        thp = P("th", 2)
        rp = P("r", 1)
        smp = P("sm", 2)
        ebp = P("eb", 2)
        fin = P("fin", 1 if mdt_name == "float32" else 2)
        drp = P("dr", 2, space="DRAM")

        # ---- weights / constants ----
        wc_sb = {}
        for d in range(2):
            for k in range(NK):
                wt = wpool.tile([128, 1200], mdt, tag=f"wc_{d}_{k}")
                nc.sync.dma_start(out=wt, in_=wc_d.ap()[d, k])
                wc_sb[(d, k)] = wt
        conv_sb = wpool.tile([128, 3], mdt, tag="conv")
        nc.sync.dma_start(out=conv_sb, in_=conv_d.ap())
        fcw_sb = wpool.tile([128, 3 * NCLS], mdt, tag="fcw")
        nc.sync.dma_start(out=fcw_sb, in_=fcw_d.ap())
        fcb_sb = wpool.tile([1, NCLS], mdt, tag="fcb")
        nc.sync.dma_start(out=fcb_sb, in_=fcb_d.ap())
        ones_sb = wpool.tile([1, 128], mdt, tag="ones")
        nc.vector.memset(ones_sb, 1.0)

        loop_cm = tc.For_i(0, repeat, 1) if repeat else None
        if loop_cm is not None:
            loop_cm.__enter__()

        # ---- persistent state ----
        # rhs k-tiles 2..4 per direction: [x tail + h | h | h + ones row]
        kt = {}
        for d in range(2):
            kt[d] = []
            for k in range(3):
                t_ = hp.tile([128, BS], mdt, tag=f"kt_{d}_{k}")
                nc.vector.memset(t_, 0.0)
                kt[d].append(t_)
            nc.sync.dma_start(out=kt[d][2][108:109], in_=ones_d.ap())  # bias row
        gc = {}    # gc[(d, j)]: [128, 1024] f32 = [tanh_g | c]
        for d in range(2):
            for j in range(3):
                g = gcp.tile([128, 1024], f32, tag=f"gc_{d}_{j}")
                nc.vector.memset(g, 0.0)
                gc[(d, j)] = g
        r = []
        for j in range(3):
            rt = rp.tile([128, BS], f32, tag=f"r_{j}")
            nc.vector.memset(rt, 0.0)
            r.append(rt)
        ssum = rp.tile([1, BS], f32, tag="ssum")
        nc.vector.memset(ssum, 0.0)

        def w_slice(d, k, col0, msz):
            return wc_sb[(d, k)][:, col0:col0 + msz]

        pending_hs = None
        tstep = [0]

        def attn_tail(hs):
            ti = tstep[0]; tstep[0] += 1
            th = []
            for j in range(3):
                thj = thp.tile([128, BS], mdt, tag=f"th{j}")
                nc.scalar.activation(out=thj, in_=hs[j], func=AF.Tanh)
                th.append(thj)
            a_ps = apool.tile([1, BS], f32, tag="a")
            for k in range(3):
                nc.tensor.matmul(a_ps, lhsT=conv_sb[:, k:k + 1], rhs=th[k],
                                 start=(k == 0), stop=(k == 2))
            if DEBUG:
                acp = smp.tile([1, BS], f32, tag="acp")
                nc.scalar.activation(out=acp, in_=a_ps, func=AF.Copy)
                nc.sync.dma_start(out=dbg_a.ap()[ti:ti+1], in_=acp)
            sg = smp.tile([1, BS], f32, tag="sg")
            nc.scalar.activation(out=sg, in_=a_ps, func=AF.Sigmoid)
            om = smp.tile([1, BS], f32, tag="om")
            nc.scalar.activation(out=om, in_=sg, func=AF.Copy, bias=1.0,
                                 scale=-1.0)
            nc.vector.reciprocal(out=om, in_=om)
            e = smp.tile([1, BS], f32, tag="e")
            nc.vector.tensor_mul(out=e, in0=sg, in1=om)   # e = exp(a)
            ed = drp.tile([1, BS], f32, tag="ed")
            nc.sync.dma_start(out=ed, in_=e)
            eb = ebp.tile([128, BS], f32, tag="eb")
            nc.sync.dma_start(out=eb, in_=ed.to_broadcast((128, BS)))
            nc.vector.tensor_add(out=ssum, in0=ssum, in1=e)
            if DEBUG:
                nc.sync.dma_start(out=dbg_e.ap()[ti:ti+1], in_=e)
            for j in range(3):
                tmp = ebp.tile([128, BS], f32, tag="rt")
                nc.gpsimd.tensor_mul(out=tmp, in0=hs[j], in1=eb)
                nc.gpsimd.tensor_add(out=r[j], in0=r[j], in1=tmp)

        # ---- time loop ----
        for t in range(T):
            x01 = []
            for k in range(2):
                xkt = xpool.tile([128, BS], mdt, tag=f"x{k}")
                nc.sync.dma_start(out=xkt, in_=xt_d.ap()[t, k])
                x01.append(xkt)
            for d in range(2):
                # x rows 256..319 (tail+pad) -> ktile2 parts 0..63
                nc.sync.dma_start(out=kt[d][0][0:64], in_=xt_d.ap()[t, 2][0:64])

            hs = []
            for j, (moff, msz, base) in enumerate(MT):
                for d in range(2):
                    rhs5 = [x01[0], x01[1], kt[d][0], kt[d][1], kt[d][2]]
                    sif = sifp.tile([128, 1024], f32, tag="sif")
                    so = sop.tile([128, BS], f32, tag="so")
                    gcj = gc[(d, j)]
                    sl = slice(base, base + msz)
                    tp = (0, base) if base else None
                    for gi, (gname, grow0) in enumerate(GATES):
                        zp = zpool.tile([128, BS], f32, tag="z")
                        zs = zp[sl]
                        for k in range(NK):
                            nc.tensor.matmul(
                                zs, lhsT=w_slice(d, k, grow0 + moff, msz),
                                rhs=rhs5[k], start=(k == 0), stop=(k == NK - 1),
                                tile_position=tp)
                        if gname == "g":
                            nc.scalar.activation(out=gcj[sl, 0:512], in_=zs,
                                                 func=AF.Tanh)
                        elif gname == "i":
                            nc.scalar.activation(out=sif[sl, 0:512], in_=zs,
                                                 func=AF.Sigmoid)
                        elif gname == "f":
                            nc.scalar.activation(out=sif[sl, 512:1024], in_=zs,
                                                 func=AF.Sigmoid)
                        else:
                            nc.scalar.activation(out=so[sl], in_=zs,
                                                 func=AF.Sigmoid)
                    # c_new = sig_f * c + sig_i * tanh_g ; h = sig_o * tanh(c_new)
                    p1 = p1p.tile([128, 1024], f32, tag="p1")
                    nc.vector.tensor_mul(out=p1[sl], in0=sif[sl], in1=gcj[sl])
                    nc.vector.tensor_add(out=gcj[sl, 512:1024],
                                         in0=p1[sl, 0:512], in1=p1[sl, 512:1024])
                    tcj = tcp.tile([128, BS], f32, tag="tc")
                    nc.scalar.activation(out=tcj[sl], in_=gcj[sl, 512:1024],
                                         func=AF.Tanh)
                    # h lands in the rhs k-tile for the next step
                    nc.vector.tensor_mul(out=kt[d][j][sl], in0=so[sl],
                                         in1=tcj[sl])
                    if d == 1:
                        # hsum snapshot before next step's gate math overwrites
                        # kt; emitted here so it clears the DVE queue early and
                        # unblocks the next step's x-tail DMA into ktile2.
                        # Full-tile: ktile2 parts 0..63 hold x junk; convp/fcw
                        # rows there are zero, so junk never reaches a dot.
                        hsj = hsp.tile([128, BS], f32, tag=f"hs{j}")
                        nc.vector.tensor_add(out=hsj, in0=kt[0][j], in1=kt[1][j])
                        hs.append(hsj)

            # attention tail for the PREVIOUS step — its score matmul and
            # e-broadcast then overlap this step's z matmuls instead of
            # stalling the PE at each step boundary.
            if pending_hs is not None:
                attn_tail(pending_hs)
            pending_hs = hs

        attn_tail(pending_hs)

        if loop_cm is not None:
            loop_cm.__exit__(None, None, None)

        # ---- tail: hStar = tanh(r / s); logits; softmax ----
        rs = smp.tile([1, BS], f32, tag="rs")
        nc.vector.reciprocal(out=rs, in_=ssum)
        rs16 = smp.tile([1, BS], mdt, tag="rs16")
        nc.scalar.activation(out=rs16, in_=rs, func=AF.Copy)
        rsb = ebpp.tile([128, BS], f32, tag="ebp")
        nc.tensor.matmul(rsb, lhsT=ones_sb, rhs=rs16, start=True, stop=True)
        if DEBUG:
            nc.sync.dma_start(out=dbg_s.ap(), in_=ssum)
            for j in range(3):
                nc.sync.dma_start(out=dbg_r.ap()[j], in_=r[j])
        hst = []
        for j in range(3):
            hn = fin.tile([128, BS], f32, tag=f"hn{j}")
            nc.vector.tensor_mul(out=hn, in0=r[j], in1=rsb)
            if DEBUG:
                nc.sync.dma_start(out=dbg_hn.ap()[j], in_=hn)
            hj = fin.tile([128, BS], mdt, tag=f"hst{j}")
            nc.scalar.activation(out=hj, in_=hn, func=AF.Tanh)
            hst.append(hj)
        for bt in range(BS // 128):
            fcp = apool.tile([128, NCLS], f32, tag="a")
            for j in range(3):
                nc.tensor.matmul(fcp, lhsT=hst[j][:, bt * 128:(bt + 1) * 128],
                                 rhs=fcw_sb[:, j * NCLS:(j + 1) * NCLS],
                                 start=(j == 0), stop=False)
            nc.tensor.matmul(fcp, lhsT=ones_sb, rhs=fcb_sb, start=False, stop=True)
            if DEBUG:
                lcp = fin.tile([128, NCLS], f32, tag="lcp")
                nc.scalar.activation(out=lcp, in_=fcp, func=AF.Copy)
                nc.sync.dma_start(out=dbg_lg.ap()[bt], in_=lcp)
            mx = fin.tile([128, 1], f32, tag="mx")
            nc.vector.reduce_max(out=mx, in_=fcp, axis=AX.X)
            nmx = fin.tile([128, 1], f32, tag="nmx")
            nc.vector.tensor_scalar_mul(out=nmx, in0=mx, scalar1=-1.0)
            ex = fin.tile([128, NCLS], f32, tag="ex")
            nc.scalar.activation(out=ex, in_=fcp, func=AF.Exp, bias=nmx)
            sm = fin.tile([128, 1], f32, tag="smm")
            nc.vector.reduce_sum(out=sm, in_=ex, axis=AX.X)
            nc.vector.reciprocal(out=sm, in_=sm)
            ot = fin.tile([128, NCLS], f32, tag="ot")
            nc.vector.tensor_scalar_mul(out=ot, in0=ex, scalar1=sm)
            nc.sync.dma_start(out=out_d.ap()[bt * 128:(bt + 1) * 128], in_=ot)

    return nc


def _prep(x, w_ih, w_hh, b_ih, b_hh, conv_w, fc_w, fc_b, np_mdt):
    """Host-side layout prep (shared across cores + per-core x shards).

    Merged contraction rows (640 = 5 k-tiles):
      0..299   x features
      320..619 h features           (h row r at combined row 320+r)
      620      bias (rhs supplies a constant-1 row; weights carry b_ih+b_hh)
    h k-layout inside tiles 2..4: parts 64.. of kt2 = h[0:64], kt3 = h[64:192],
    kt4[0:108] = h[192:300], kt4[108] = ones.
    """
    bias = (b_ih + b_hh).astype(np.float32)  # [2, 1200]
    wc = np.zeros((2, NK, 128, 1200), np.float32)
    for d in range(2):
        comb = np.zeros((KP, 1200), np.float32)
        comb[XOFF:XOFF + H] = w_ih[d].T
        comb[HOFF:HOFF + H] = w_hh[d].T
        comb[BROW] = bias[d]
        wc[d] = comb.reshape(NK, 128, 1200)

    def h_pack(vec_or_mat, width):
        """Pack [300(, width)] h-feature data into the 3-tile h k-layout."""
        out = np.zeros((3, 128, width), np.float32)
        v = vec_or_mat.reshape(H, width)
        out[0, 64:128] = v[0:64]
        out[1, :] = v[64:192]
        out[2, 0:108] = v[192:300]
        return out

    convp = np.ascontiguousarray(
        h_pack(conv_w, 1).reshape(3, 128).T)          # [128, 3]
    fcw = np.ascontiguousarray(
        h_pack(fc_w.T, NCLS).transpose(1, 0, 2).reshape(128, 3 * NCLS))

    shared = {
        "wc": wc.astype(np_mdt),
        "convp": convp.astype(np_mdt),
        "fcw": fcw.astype(np_mdt),
        "fcb": fc_b.reshape(1, NCLS).astype(np_mdt),
        "onesrow": np.ones((1, BS), np.float32).astype(np_mdt),
    }

    # x: [B, H, T] -> per-core [T, 3, 128, BS]; tile2 rows 300..383 are zero
    # (device DMAs only rows 256..319 of it into ktile2 parts 0..63).
    xs = np.ascontiguousarray(np.transpose(x, (2, 1, 0)))  # [T, H, B]
    xp = np.zeros((T, 384, B), np.float32)
    xp[:, :H] = xs
    xp = xp.reshape(T, 3, 128, NCORES, BS)
    in_maps = []
    for c in range(NCORES):
        m = dict(shared)
        m["xt"] = np.ascontiguousarray(xp[:, :, :, c]).astype(np_mdt)
        in_maps.append(m)
    return in_maps


def kernel(x, w_ih, w_hh, b_ih, b_hh, conv_w, fc_w, fc_b):
    global LAST_EXEC_NS
    mdt_name = MM_DT_NAME
    np_mdt = np.float16 if mdt_name == "float16" else (
        __import__("ml_dtypes").bfloat16 if mdt_name == "bfloat16" else np.float32)
    if mdt_name not in _CACHE:
        _CACHE[mdt_name] = _Runner(_build(mdt_name), NCORES)
    runner = _CACHE[mdt_name]
    in_maps = _prep(np.asarray(x, np.float32), np.asarray(w_ih, np.float32),
                    np.asarray(w_hh, np.float32), np.asarray(b_ih, np.float32),
                    np.asarray(b_hh, np.float32), np.asarray(conv_w, np.float32),
                    np.asarray(fc_w, np.float32), np.asarray(fc_b, np.float32),
                    np_mdt)
    results = runner.run(in_maps)
    out = np.concatenate([r["out"] for r in results], axis=0)
    return out.astype(np.float32)


def bench(x, w_ih, w_hh, b_ih, b_hh, conv_w, fc_w, fc_b, iters=5):
    mdt_name = MM_DT_NAME
    np_mdt = np.float16 if mdt_name == "float16" else (
        __import__("ml_dtypes").bfloat16 if mdt_name == "bfloat16" else np.float32)
    if mdt_name not in _CACHE:
        _CACHE[mdt_name] = _Runner(_build(mdt_name), NCORES)
    runner = _CACHE[mdt_name]
    in_maps = _prep(np.asarray(x, np.float32), np.asarray(w_ih, np.float32),
                    np.asarray(w_hh, np.float32), np.asarray(b_ih, np.float32),
                    np.asarray(b_hh, np.float32), np.asarray(conv_w, np.float32),
                    np.asarray(fc_w, np.float32), np.asarray(fc_b, np.float32),
                    np_mdt)
    return runner.bench(in_maps, iters=iters)



# revision 2
# speedup vs baseline: 44.7038x; 44.7038x over previous
"""AttentionLSTM Trainium2 kernel — 8-core data-parallel.

Model (per batch row b): two independent single-direction LSTMs over T=43
steps of x[:, :, t] (H=300 features), hidden states summed, then a
conv-softmax attention over time, tanh, fc(300->80), softmax.

Device mapping per core (512 batch rows):
  - z^T[1200, 512] per (direction, step) via PE matmuls: merged contraction
    K=5 k-tiles of 128 (h rows 0..299 first, then bias + x tail, then x),
    M gate-aligned tiles {128,128,44}, fp16 MM inputs, fp32 PSUM accum.
  - h k-tiles are parity double-buffered: step t reads kt[t%2], writes h_t
    into kt[(t+1)%2], so every matmul of a step sees the full h_{t-1}
    (exact LSTM semantics, no Gauss-Seidel staleness).
  - group order is d-major (all of direction 0, then direction 1): d0's
    elementwise tail hides under d1's matmuls and vice versa across the
    step boundary, keeping the PE dense.
  - gates: one fused sigmoid over an [*,1024] i|f PSUM pair, tanh(g) and
    sigmoid(o) over a shared g|o PSUM pair; gate/cell elementwise state in
    fp16 for 2x DVE throughput; c stays in SBUF.
  - attention accumulated online: e_t = sigmoid(a)/(1-sigmoid(a)) = exp(a)
    (avoids exp table loads mid-loop); e_t broadcast across partitions with
    a rank-1 PE matmul (ones x e) into PSUM — no DRAM round trip; r += on
    GPSIMD.
  - tail: hStar = tanh(r/s), logits = fc(hStar) via PE (batch on PSUM
    partitions), softmax over the 80-class free dim.
"""

import os
import sys

sys.path.insert(0, "/opt/trn_rl_repo")

from contextlib import ExitStack

import numpy as np

import concourse.bass as bass
import concourse.tile as tile
from concourse import mybir
from concourse.bass_utils import run_bass_kernel_spmd  # noqa: F401  (spmd path kept available)

f32 = mybir.dt.float32
AF = mybir.ActivationFunctionType
AX = mybir.AxisListType

_BIRFIX_DONE = False


def _split_multiwaits(bir_json):
    """This walrus build allows one sync-wait per engine instruction; Tile
    attaches one per producer proc. Hoist extras onto standalone
    EventSemaphore instructions inserted just before, same engine queue."""
    import json
    j = json.loads(bir_json.decode() if isinstance(bir_json, bytes) else bir_json)
    for fn in j.get("functions", []):
        for blk in fn.get("blocks", []):
            out = []
            for ins in blk.get("instructions", []):
                si = ins.get("sync_info")
                ow = si.get("on_wait") if si else None
                if ow and len(ow) > 1:
                    for i, w in enumerate(ow[:-1]):
                        out.append({
                            "debug": ins.get("debug", 0),
                            "engine": ins["engine"],
                            "ins": [], "outs": [],
                            "name": f"{ins['name']}_xw{i}",
                            "opcode": "EventSemaphore",
                            "sync_info": {"on_update": [], "on_wait": [w]},
                        })
                    si["on_wait"] = [ow[-1]]
                out.append(ins)
            blk["instructions"] = out
    return json.dumps(j).encode()


def _install_birfix():
    global _BIRFIX_DONE
    if _BIRFIX_DONE:
        return
    from concourse import bass2jax
    orig = bass2jax.compile_bir_kernel

    def patched(bir_json, tmpdir, neff_name="file.neff"):
        return orig(_split_multiwaits(bir_json), tmpdir, neff_name)

    bass2jax.compile_bir_kernel = patched
    _BIRFIX_DONE = True


class _Runner:
    """Compile once; keep the sharded jitted executable + device inputs."""

    def __init__(self, nc, n_cores):
        import jax
        from jax.sharding import Mesh, PartitionSpec
        from jax.experimental.shard_map import shard_map
        from concourse import bass2jax as b2j

        b2j.install_neuronx_cc_hook()
        _install_birfix()
        self.jax = jax
        self.nc = nc
        self.n_cores = n_cores
        part_name = nc.partition_id_tensor.name if nc.partition_id_tensor else None
        in_names, out_names, out_avals, zero_outs = [], [], [], []
        for alloc in nc.m.functions[0].allocations:
            if not isinstance(alloc, mybir.MemoryLocationSet):
                continue
            name = alloc.memorylocations[0].name
            if alloc.kind == "ExternalInput":
                if name != part_name:
                    in_names.append(name)
            elif alloc.kind == "ExternalOutput":
                out_names.append(name)
                shape = tuple(alloc.tensor_shape)
                dtype = mybir.dt.np(alloc.dtype)
                out_avals.append(jax.core.ShapedArray(shape, dtype))
                zero_outs.append(np.zeros(shape, dtype))
        self.in_names = list(in_names)
        self.out_names = out_names
        self.out_avals = out_avals
        self.zero_outs = zero_outs
        n_params = len(in_names)
        n_outs = len(out_avals)
        all_names = in_names + out_names
        if part_name is not None:
            all_names = all_names + [part_name]
        donate = tuple(range(n_params, n_params + n_outs))

        def _body(*args):
            operands = list(args)
            if part_name is not None:
                operands.append(b2j.partition_id_tensor())
            outs = b2j._bass_exec_p.bind(
                *operands,
                out_avals=tuple(out_avals),
                in_names=tuple(all_names),
                out_names=tuple(out_names),
                lowering_input_output_aliases=(),
                sim_require_finite=True,
                sim_require_nnan=True,
                nc=nc,
            )
            return tuple(outs)

        devices = jax.devices()[:n_cores]
        self.mesh = Mesh(np.asarray(devices), ("core",))
        in_specs = (PartitionSpec("core"),) * (n_params + n_outs)
        out_specs = (PartitionSpec("core"),) * n_outs
        self.sharded = jax.jit(
            shard_map(_body, mesh=self.mesh, in_specs=in_specs,
                      out_specs=out_specs, check_rep=False),
            donate_argnums=donate, keep_unused=True)
        self.sharding = jax.sharding.NamedSharding(
            self.mesh, PartitionSpec("core"))

    def put_inputs(self, in_maps):
        jax = self.jax
        concat = [np.concatenate([np.asarray(m[n]) for m in in_maps], axis=0)
                  for n in self.in_names]
        return [jax.device_put(a, self.sharding) for a in concat]

    def call(self, dev_in):
        zeros = [np.zeros((self.n_cores * z.shape[0], *z.shape[1:]), z.dtype)
                 for z in self.zero_outs]
        outs = self.sharded(*dev_in, *zeros)
        self.jax.block_until_ready(outs)
        return outs

    def run(self, in_maps):
        dev_in = self.put_inputs(in_maps)
        outs = self.call(dev_in)
        n = self.n_cores
        return [
            {name: np.asarray(outs[i]).reshape(n, *self.out_avals[i].shape)[c]
             for i, name in enumerate(self.out_names)}
            for c in range(n)
        ]

    def bench(self, in_maps, iters=5):
        import time
        dev_in = self.put_inputs(in_maps)
        self.call(dev_in)  # warm
        times = []
        for _ in range(iters):
            t0 = time.perf_counter()
            self.call(dev_in)
            times.append(time.perf_counter() - t0)
        return times


B, H, T, NCLS = 4096, 300, 43, 80
NCORES = 8
BS = B // NCORES          # 512 batch rows per core
NK = 5                    # k-tiles: [h0:128 | h128:256 | h256:300+bias+xtail | x0:128 | x128:256]
BIASROW = 44              # partition of the bias (constant-1) row in k-tile 2
XTAIL = 64                # x rows 256..300 live at parts 64..108 of k-tile 2
MT = [(0, 128), (128, 128), (256, 44)]    # (moff, msz) per gate, output base partition 0
GOFF = [0, 300, 600, 900]                 # torch gate order i,f,g,o

MM_DT_NAME = os.environ.get("LSTM_MM_DT", "float16")

_CACHE = {}


def _build(mdt_name, repeat=0):
    mdt = getattr(mybir.dt, mdt_name)
    nc = bass.Bass(target_bir_lowering=False)

    xt_d = nc.declare_dram_parameter("xt", [T, 3, 128, BS], mdt, isOutput=False)
    wc_d = nc.declare_dram_parameter("wc", [2, NK, 128, 1200], mdt, isOutput=False)
    conv_d = nc.declare_dram_parameter("convp", [128, 3], mdt, isOutput=False)
    fcw_d = nc.declare_dram_parameter("fcw", [128, 3 * NCLS], mdt, isOutput=False)
    fcb_d = nc.declare_dram_parameter("fcb", [1, NCLS], mdt, isOutput=False)
    ones_d = nc.declare_dram_parameter("onesrow", [1, BS], mdt, isOutput=False)
    out_d = nc.declare_dram_parameter("out", [BS, NCLS], f32, isOutput=True)

    with tile.TileContext(nc) as tc, ExitStack() as ctx:
        P = lambda name, bufs, **kw: ctx.enter_context(
            tc.tile_pool(name=name, bufs=bufs, **kw))
        wpool = P("w", 1)
        xpool = P("x", 3)
        zifp = P("zif", 2, space="PSUM")    # [128,1024] f32 -> 2 banks x 2
        zgop = P("zgo", 1, space="PSUM")    # [128,1024] f32 -> 2 banks
        apsp = P("aps", 1, space="PSUM")    # [1,512] -> 1 bank (reused by fc tail)
        ebpp = P("ebps", 1, space="PSUM")   # [128,512] -> 1 bank
        sifp = P("sif", 3)
        sop = P("so", 3)
        gcp = P("gc", 1)
        p1p = P("p1", 3)
        tcp = P("tc", 3)
        hp = P("h", 1)
        hsp = P("hs", 2)
        thp = P("th", 2)
        rp = P("r", 1)
        smp = P("sm", 2)
        tmpp = P("tmp", 2)
        fin = P("fin", 2)

        # ---- weights / constants ----
        wc_sb = {}
        for d in range(2):
            for k in range(NK):
                wt = wpool.tile([128, 1200], mdt, tag=f"wc_{d}_{k}")
                nc.sync.dma_start(out=wt, in_=wc_d.ap()[d, k])
                wc_sb[(d, k)] = wt
        conv_sb = wpool.tile([128, 3], mdt, tag="conv")
        nc.sync.dma_start(out=conv_sb, in_=conv_d.ap())
        fcw_sb = wpool.tile([128, 3 * NCLS], mdt, tag="fcw")
        nc.sync.dma_start(out=fcw_sb, in_=fcw_d.ap())
        fcb_sb = wpool.tile([1, NCLS], mdt, tag="fcb")
        nc.sync.dma_start(out=fcb_sb, in_=fcb_d.ap())
        ones_sb = wpool.tile([1, 128], mdt, tag="ones")
        nc.vector.memset(ones_sb, 1.0)

        # ---- persistent state ----
        # h k-tiles, parity double-buffered: step t reads kt[t%2][d],
        # writes h_t into kt[(t+1)%2][d].
        kt = {}
        for par in range(2):
            for d in range(2):
                kt[(par, d)] = []
                for j in range(3):
                    t_ = hp.tile([128, BS], mdt, tag=f"kt_{par}_{d}_{j}")
                    nc.vector.memset(t_, 0.0)
                    kt[(par, d)].append(t_)
                nc.sync.dma_start(out=kt[(par, d)][2][BIASROW:BIASROW + 1],
                                  in_=ones_d.ap())
        gc = {}    # gc[(d, j)]: [128, 1024] mdt = [tanh_g | c]
        for d in range(2):
            for j in range(3):
                g = gcp.tile([128, 1024], mdt, tag=f"gc_{d}_{j}")
                nc.vector.memset(g, 0.0)
                gc[(d, j)] = g
        r = []
        for j in range(3):
            rt = rp.tile([128, BS], f32, tag=f"r_{j}")
            nc.vector.memset(rt, 0.0)
            r.append(rt)
        ssum = rp.tile([1, BS], f32, tag="ssum")
        nc.vector.memset(ssum, 0.0)

        def w_slice(d, k, col0, msz):
            return wc_sb[(d, k)][:, col0:col0 + msz]

        def attn_tail(hs):
            # hs[j]: [128, BS] mdt hsum tiles from the PREVIOUS step.
            th = []
            for j in range(3):
                pmax = 45 if j == 2 else 128
                thj = thp.tile([128, BS], mdt, tag=f"th{j}")
                nc.scalar.activation(out=thj[0:pmax], in_=hs[j][0:pmax],
                                     func=AF.Tanh)
                th.append((thj, pmax))
            a_ps = apsp.tile([1, BS], f32, tag="a")
            for k in range(3):
                thj, pmax = th[k]
                nc.tensor.matmul(a_ps, lhsT=conv_sb[0:pmax, k:k + 1],
                                 rhs=thj[0:pmax], start=(k == 0), stop=(k == 2))
            sg = smp.tile([1, BS], f32, tag="sg")
            nc.scalar.activation(out=sg, in_=a_ps, func=AF.Sigmoid)
            om = smp.tile([1, BS], f32, tag="om")
            nc.scalar.activation(out=om, in_=sg, func=AF.Copy, bias=1.0,
                                 scale=-1.0)
            nc.vector.reciprocal(out=om, in_=om)
            e = smp.tile([1, BS], f32, tag="e")
            nc.vector.tensor_mul(out=e, in0=sg, in1=om)   # e = exp(a)
            nc.vector.tensor_add(out=ssum, in0=ssum, in1=e)
            e16 = smp.tile([1, BS], mdt, tag="e16")
            nc.scalar.activation(out=e16, in_=e, func=AF.Copy)
            eb_ps = ebpp.tile([128, BS], f32, tag="eb")
            nc.tensor.matmul(eb_ps, lhsT=ones_sb, rhs=e16, start=True, stop=True)
            for j in range(3):
                pmax = 45 if j == 2 else 128
                tmp = tmpp.tile([128, BS], f32, tag=f"tmp{j}")
                nc.vector.tensor_mul(out=tmp[0:pmax], in0=hs[j][0:pmax],
                                     in1=eb_ps[0:pmax])
                nc.gpsimd.tensor_add(out=r[j][0:pmax], in0=r[j][0:pmax],
                                     in1=tmp[0:pmax])

        loop_cm = tc.For_i(0, repeat, 1) if repeat else None
        if loop_cm is not None:
            loop_cm.__enter__()

        pending_hs = None

        # ---- time loop ----
        for t in range(T):
            par, nxt = t % 2, (t + 1) % 2
            xa = xpool.tile([128, BS], mdt, tag="xa")
            nc.sync.dma_start(out=xa, in_=xt_d.ap()[t, 0])
            xb = xpool.tile([128, BS], mdt, tag="xb")
            nc.sync.dma_start(out=xb, in_=xt_d.ap()[t, 1])
            for d in range(2):
                nc.sync.dma_start(out=kt[(par, d)][2][XTAIL:XTAIL + 44],
                                  in_=xt_d.ap()[t, 2][XTAIL:XTAIL + 44])

            hs = []
            for d in range(2):
                rhs5 = [kt[(par, d)][0], kt[(par, d)][1], kt[(par, d)][2],
                        xa, xb]
                for j, (moff, msz) in enumerate(MT):
                    sl = slice(0, msz)
                    zif = zifp.tile([128, 1024], f32, tag="zif")
                    zgo = zgop.tile([128, 1024], f32, tag="zgo")
                    for gi, zdst in ((0, zif[sl, 0:512]),
                                     (1, zif[sl, 512:1024]),
                                     (2, zgo[sl, 0:512]),
                                     (3, zgo[sl, 512:1024])):
                        col0 = GOFF[gi] + moff
                        for k in range(NK):
                            nc.tensor.matmul(
                                zdst, lhsT=w_slice(d, k, col0, msz),
                                rhs=rhs5[k], start=(k == 0), stop=(k == NK - 1))
                    sif = sifp.tile([128, 1024], mdt, tag="sif")
                    nc.scalar.activation(out=sif[sl], in_=zif[sl],
                                         func=AF.Sigmoid)
                    gcj = gc[(d, j)]
                    nc.scalar.activation(out=gcj[sl, 0:512], in_=zgo[sl, 0:512],
                                         func=AF.Tanh)
                    so = sop.tile([128, BS], mdt, tag="so")
                    nc.scalar.activation(out=so[sl], in_=zgo[sl, 512:1024],
                                         func=AF.Sigmoid)
                    # c_new = sig_f * c + sig_i * tanh_g ; h = sig_o * tanh(c)
                    p1 = p1p.tile([128, 1024], mdt, tag="p1")
                    nc.vector.tensor_mul(out=p1[sl], in0=sif[sl], in1=gcj[sl])
                    nc.vector.tensor_add(out=gcj[sl, 512:1024],
                                         in0=p1[sl, 0:512], in1=p1[sl, 512:1024])
                    tcj = tcp.tile([128, BS], mdt, tag="tc")
                    nc.scalar.activation(out=tcj[sl], in_=gcj[sl, 512:1024],
                                         func=AF.Tanh)
                    # h_t lands directly in the next step's rhs k-tile
                    nc.vector.tensor_mul(out=kt[(nxt, d)][j][sl], in0=so[sl],
                                         in1=tcj[sl])
                    if d == 1:
                        pmax = 45 if j == 2 else 128
                        hsj = hsp.tile([128, BS], mdt, tag=f"hs{j}")
                        nc.vector.tensor_add(out=hsj[0:pmax],
                                             in0=kt[(nxt, 0)][j][0:pmax],
                                             in1=kt[(nxt, 1)][j][0:pmax])
                        hs.append(hsj)
                if d == 0 and pending_hs is not None:
                    # attention tail for the PREVIOUS step: its PE/ACT/DVE
                    # work overlaps this step's d1 matmul phase.
                    attn_tail(pending_hs)
            pending_hs = hs

        attn_tail(pending_hs)

        if loop_cm is not None:
            loop_cm.__exit__(None, None, None)

        # ---- tail: hStar = tanh(r / s); logits; softmax ----
        rs = smp.tile([1, BS], f32, tag="rs")
        nc.vector.reciprocal(out=rs, in_=ssum)
        rs16 = smp.tile([1, BS], mdt, tag="rs16")
        nc.scalar.activation(out=rs16, in_=rs, func=AF.Copy)
        rsb = ebpp.tile([128, BS], f32, tag="eb")
        nc.tensor.matmul(rsb, lhsT=ones_sb, rhs=rs16, start=True, stop=True)
        hst = []
        for j in range(3):
            hn = fin.tile([128, BS], f32, tag=f"hn{j}")
            nc.vector.tensor_mul(out=hn, in0=r[j], in1=rsb)
            hj = fin.tile([128, BS], mdt, tag=f"hst{j}")
            nc.scalar.activation(out=hj, in_=hn, func=AF.Tanh)
            hst.append(hj)
        for bt in range(BS // 128):
            fcp = apsp.tile([128, NCLS], f32, tag="a")
            for j in range(3):
                nc.tensor.matmul(fcp, lhsT=hst[j][:, bt * 128:(bt + 1) * 128],
                                 rhs=fcw_sb[:, j * NCLS:(j + 1) * NCLS],
                                 start=(j == 0), stop=False)
            nc.tensor.matmul(fcp, lhsT=ones_sb, rhs=fcb_sb, start=False, stop=True)
            mx = fin.tile([128, 1], f32, tag="mx")
            nc.vector.reduce_max(out=mx, in_=fcp, axis=AX.X)
            nmx = fin.tile([128, 1], f32, tag="nmx")
            nc.vector.tensor_scalar_mul(out=nmx, in0=mx, scalar1=-1.0)
            ex = fin.tile([128, NCLS], f32, tag="ex")
            nc.scalar.activation(out=ex, in_=fcp, func=AF.Exp, bias=nmx)
            sm = fin.tile([128, 1], f32, tag="smm")
            nc.vector.reduce_sum(out=sm, in_=ex, axis=AX.X)
            nc.vector.reciprocal(out=sm, in_=sm)
            ot = fin.tile([128, NCLS], f32, tag="ot")
            nc.vector.tensor_scalar_mul(out=ot, in0=ex, scalar1=sm)
            nc.sync.dma_start(out=out_d.ap()[bt * 128:(bt + 1) * 128], in_=ot)

    return nc


def _prep(x, w_ih, w_hh, b_ih, b_hh, conv_w, fc_w, fc_b, np_mdt):
    """Host-side layout prep (shared across cores + per-core x shards).

    Merged contraction rows (640 = 5 k-tiles of 128):
      tile 0: h[0:128]        tile 1: h[128:256]
      tile 2: h[256:300] at parts 0..43, bias (const-1 row) at part 44,
              x[256:300] at parts 64..107, zeros elsewhere
      tile 3: x[0:128]        tile 4: x[128:256]
    """
    bias = (b_ih + b_hh).astype(np.float32)  # [2, 1200]
    wc = np.zeros((2, NK, 128, 1200), np.float32)
    for d in range(2):
        comb = np.zeros((NK * 128, 1200), np.float32)
        comb[0:256] = w_hh[d].T[0:256]
        comb[256:300] = w_hh[d].T[256:300]
        comb[256 + BIASROW] = bias[d]
        comb[256 + XTAIL:256 + XTAIL + 44] = w_ih[d].T[256:300]
        comb[384:512] = w_ih[d].T[0:128]
        comb[512:640] = w_ih[d].T[128:256]
        wc[d] = comb.reshape(NK, 128, 1200)

    def h_pack(vec_or_mat, width):
        """Pack [300(, width)] h-feature data into the 3-tile h k-layout."""
        out = np.zeros((3, 128, width), np.float32)
        v = vec_or_mat.reshape(H, width)
        out[0] = v[0:128]
        out[1] = v[128:256]
        out[2, 0:44] = v[256:300]
        return out

    convp = np.ascontiguousarray(
        h_pack(conv_w, 1).reshape(3, 128).T)          # [128, 3]
    fcw = np.ascontiguousarray(
        h_pack(fc_w.T, NCLS).transpose(1, 0, 2).reshape(128, 3 * NCLS))

    shared = {
        "wc": wc.astype(np_mdt),
        "convp": convp.astype(np_mdt),
        "fcw": fcw.astype(np_mdt),
        "fcb": fc_b.reshape(1, NCLS).astype(np_mdt),
        "onesrow": np.ones((1, BS), np.float32).astype(np_mdt),
    }

    # x: [B, H, T] -> per-core [T, 3, 128, BS]:
    # slot 0 = x[0:128], slot 1 = x[128:256],
    # slot 2 = zeros with x[256:300] at parts 64..107.
    xs = np.ascontiguousarray(np.transpose(x, (2, 1, 0)))  # [T, H, B]
    xp = np.zeros((T, 3, 128, B), np.float32)
    xp[:, 0] = xs[:, 0:128]
    xp[:, 1] = xs[:, 128:256]
    xp[:, 2, XTAIL:XTAIL + 44] = xs[:, 256:300]
    xp = xp.reshape(T, 3, 128, NCORES, BS)
    in_maps = []
    for c in range(NCORES):
        m = dict(shared)
        m["xt"] = np.ascontiguousarray(xp[:, :, :, c]).astype(np_mdt)
        in_maps.append(m)
    return in_maps


def _np_mdt(mdt_name):
    return np.float16 if mdt_name == "float16" else (
        __import__("ml_dtypes").bfloat16 if mdt_name == "bfloat16" else np.float32)


def _runner(repeat=0):
    key = (MM_DT_NAME, repeat)
    if key not in _CACHE:
        _CACHE[key] = _Runner(_build(MM_DT_NAME, repeat=repeat), NCORES)
    return _CACHE[key]


def _in_maps(inputs_f32):
    return _prep(*inputs_f32, _np_mdt(MM_DT_NAME))


def _inputs_f32(x, w_ih, w_hh, b_ih, b_hh, conv_w, fc_w, fc_b):
    return [np.asarray(a, np.float32) for a in
            (x, w_ih, w_hh, b_ih, b_hh, conv_w, fc_w, fc_b)]


def kernel(x, w_ih, w_hh, b_ih, b_hh, conv_w, fc_w, fc_b):
    runner = _runner(repeat=0)
    in_maps = _in_maps(_inputs_f32(x, w_ih, w_hh, b_ih, b_hh,
                                   conv_w, fc_w, fc_b))
    results = runner.run(in_maps)
    out = np.concatenate([r["out"] for r in results], axis=0)
    return out.astype(np.float32)


def bench(x, w_ih, w_hh, b_ih, b_hh, conv_w, fc_w, fc_b, iters=5):
    runner = _runner(repeat=0)
    in_maps = _in_maps(_inputs_f32(x, w_ih, w_hh, b_ih, b_hh,
                                   conv_w, fc_w, fc_b))
    return runner.bench(in_maps, iters=iters)


def measure_exec_ns(inputs, r_lo=1, r_hi=41, iters=8):
    """Device execution time of one full forward pass, in ns.

    The axon tunnel adds a fixed ~70-80 ms completion-notification latency
    to every blocking call, independent of what the NEFF does (measured:
    a trivial 4-instruction kernel takes the same wall time as the full
    LSTM).  To measure hardware execution, both builds wrap the whole
    T-step forward in a hardware For_i loop (r_lo vs r_hi iterations,
    identical instruction stream per iteration); the slope
    (min_wall(r_hi) - min_wall(r_lo)) / (r_hi - r_lo) is the steady-state
    on-device time of one forward pass with the constant latency cancelled.
    Samples are interleaved so network drift affects both arms equally.
    """
    import time
    in_maps = _in_maps(_inputs_f32(**inputs) if isinstance(inputs, dict)
                       else _inputs_f32(*inputs))
    runners = {rep: _runner(repeat=rep) for rep in (r_lo, r_hi)}
    dev_in = {rep: runners[rep].put_inputs(in_maps) for rep in (r_lo, r_hi)}
    for rep in (r_lo, r_hi):
        runners[rep].call(dev_in[rep])  # warm
    walls = {r_lo: [], r_hi: []}
    for _ in range(iters):
        for rep in (r_lo, r_hi):
            t0 = time.perf_counter()
            runners[rep].call(dev_in[rep])
            walls[rep].append(time.perf_counter() - t0)
    lo, hi = min(walls[r_lo]), min(walls[r_hi])
    ns = (hi - lo) * 1e9 / (r_hi - r_lo)
    return max(int(ns), 1), walls


# revision 10
# speedup vs baseline: 68.2332x; 1.5263x over previous
"""AttentionLSTM Trainium2 kernel — 8-core data-parallel.

Model (per batch row b): two independent single-direction LSTMs over T=43
steps of x[:, :, t] (H=300 features), hidden states summed, then a
conv-softmax attention over time, tanh, fc(300->80), softmax.

Device mapping per core (512 batch rows):
  - z^T[1200, 512] per (direction, step) via PE matmuls: merged contraction
    K=5 k-tiles of 128 (h rows 0..299 first, then bias + x tail, then x),
    M gate-aligned tiles {128,128,44}, fp16 MM inputs, fp32 PSUM accum.
  - h k-tiles are parity double-buffered: step t reads kt[t%2], writes h_t
    into kt[(t+1)%2], so every matmul of a step sees the full h_{t-1}
    (exact LSTM semantics, no Gauss-Seidel staleness).
  - group order is d-major (all of direction 0, then direction 1): d0's
    elementwise tail hides under d1's matmuls and vice versa across the
    step boundary, keeping the PE dense.
  - gates: one fused sigmoid over an [*,1024] i|f PSUM pair, tanh(g) and
    sigmoid(o) over a shared g|o PSUM pair; gate/cell elementwise state in
    fp16 for 2x DVE throughput; c stays in SBUF.
  - attention accumulated online: e_t = sigmoid(a)/(1-sigmoid(a)) = exp(a)
    (avoids exp table loads mid-loop); e_t broadcast across partitions with
    a rank-1 PE matmul (ones x e) into PSUM — no DRAM round trip; r += on
    GPSIMD.
  - tail: hStar = tanh(r/s), logits = fc(hStar) via PE (batch on PSUM
    partitions), softmax over the 80-class free dim.
"""

import os
import sys

sys.path.insert(0, "/opt/trn_rl_repo")

from contextlib import ExitStack

import numpy as np

import concourse.bass as bass
import concourse.tile as tile
from concourse import mybir
from concourse.bass_utils import run_bass_kernel_spmd  # noqa: F401  (spmd path kept available)

f32 = mybir.dt.float32
AF = mybir.ActivationFunctionType
AX = mybir.AxisListType

_BIRFIX_DONE = False


def _split_multiwaits(bir_json):
    """This walrus build allows one sync-wait per engine instruction; Tile
    attaches one per producer proc. Hoist extras onto standalone
    EventSemaphore instructions inserted just before, same engine queue."""
    import json
    j = json.loads(bir_json.decode() if isinstance(bir_json, bytes) else bir_json)
    for fn in j.get("functions", []):
        for blk in fn.get("blocks", []):
            out = []
            for ins in blk.get("instructions", []):
                si = ins.get("sync_info")
                ow = si.get("on_wait") if si else None
                if ow and len(ow) > 1:
                    for i, w in enumerate(ow[:-1]):
                        out.append({
                            "debug": ins.get("debug", 0),
                            "engine": ins["engine"],
                            "ins": [], "outs": [],
                            "name": f"{ins['name']}_xw{i}",
                            "opcode": "EventSemaphore",
                            "sync_info": {"on_update": [], "on_wait": [w]},
                        })
                    si["on_wait"] = [ow[-1]]
                out.append(ins)
            blk["instructions"] = out
    return json.dumps(j).encode()


def _install_birfix():
    global _BIRFIX_DONE
    if _BIRFIX_DONE:
        return
    from concourse import bass2jax
    orig = bass2jax.compile_bir_kernel

    def patched(bir_json, tmpdir, neff_name="file.neff"):
        return orig(_split_multiwaits(bir_json), tmpdir, neff_name)

    bass2jax.compile_bir_kernel = patched
    _BIRFIX_DONE = True


class _Runner:
    """Compile once; keep the sharded jitted executable + device inputs."""

    def __init__(self, nc, n_cores):
        import jax
        from jax.sharding import Mesh, PartitionSpec
        from jax.experimental.shard_map import shard_map
        from concourse import bass2jax as b2j

        b2j.install_neuronx_cc_hook()
        _install_birfix()
        self.jax = jax
        self.nc = nc
        self.n_cores = n_cores
        part_name = nc.partition_id_tensor.name if nc.partition_id_tensor else None
        in_names, out_names, out_avals, zero_outs = [], [], [], []
        for alloc in nc.m.functions[0].allocations:
            if not isinstance(alloc, mybir.MemoryLocationSet):
                continue
            name = alloc.memorylocations[0].name
            if alloc.kind == "ExternalInput":
                if name != part_name:
                    in_names.append(name)
            elif alloc.kind == "ExternalOutput":
                out_names.append(name)
                shape = tuple(alloc.tensor_shape)
                dtype = mybir.dt.np(alloc.dtype)
                out_avals.append(jax.core.ShapedArray(shape, dtype))
                zero_outs.append(np.zeros(shape, dtype))
        self.in_names = list(in_names)
        self.out_names = out_names
        self.out_avals = out_avals
        self.zero_outs = zero_outs
        n_params = len(in_names)
        n_outs = len(out_avals)
        all_names = in_names + out_names
        if part_name is not None:
            all_names = all_names + [part_name]
        donate = tuple(range(n_params, n_params + n_outs))

        def _body(*args):
            operands = list(args)
            if part_name is not None:
                operands.append(b2j.partition_id_tensor())
            outs = b2j._bass_exec_p.bind(
                *operands,
                out_avals=tuple(out_avals),
                in_names=tuple(all_names),
                out_names=tuple(out_names),
                lowering_input_output_aliases=(),
                sim_require_finite=True,
                sim_require_nnan=True,
                nc=nc,
            )
            return tuple(outs)

        devices = jax.devices()[:n_cores]
        self.mesh = Mesh(np.asarray(devices), ("core",))
        in_specs = (PartitionSpec("core"),) * (n_params + n_outs)
        out_specs = (PartitionSpec("core"),) * n_outs
        self.sharded = jax.jit(
            shard_map(_body, mesh=self.mesh, in_specs=in_specs,
                      out_specs=out_specs, check_rep=False),
            donate_argnums=donate, keep_unused=True)
        self.sharding = jax.sharding.NamedSharding(
            self.mesh, PartitionSpec("core"))

    def put_inputs(self, in_maps):
        jax = self.jax
        concat = [np.concatenate([np.asarray(m[n]) for m in in_maps], axis=0)
                  for n in self.in_names]
        return [jax.device_put(a, self.sharding) for a in concat]

    def call(self, dev_in):
        zeros = [np.zeros((self.n_cores * z.shape[0], *z.shape[1:]), z.dtype)
                 for z in self.zero_outs]
        outs = self.sharded(*dev_in, *zeros)
        self.jax.block_until_ready(outs)
        return outs

    def run(self, in_maps):
        dev_in = self.put_inputs(in_maps)
        outs = self.call(dev_in)
        n = self.n_cores
        return [
            {name: np.asarray(outs[i]).reshape(n, *self.out_avals[i].shape)[c]
             for i, name in enumerate(self.out_names)}
            for c in range(n)
        ]

    def bench(self, in_maps, iters=5):
        import time
        dev_in = self.put_inputs(in_maps)
        self.call(dev_in)  # warm
        times = []
        for _ in range(iters):
            t0 = time.perf_counter()
            self.call(dev_in)
            times.append(time.perf_counter() - t0)
        return times


B, H, T, NCLS = 4096, 300, 43, 80
NCORES = 8
BS = B // NCORES          # 512 batch rows per core
NK = 5                    # k-tiles: [h0:128 | h128:256 | h256:300+bias+xtail | x0:128 | x128:256]
BIASROW = 44              # partition of the bias (constant-1) row in k-tile 2
XTAIL = 64                # x rows 256..300 live at parts 64..108 of k-tile 2
MT = [(0, 128), (128, 128), (256, 44)]    # (moff, msz) per gate, output base partition 0
GOFF = [0, 300, 600, 900]                 # torch gate order i,f,g,o

MM_DT_NAME = os.environ.get("LSTM_MM_DT", "float16")

_CACHE = {}


def _build(mdt_name, repeat=0, variant="full"):
    # variant: "full" | "no_attn" (skip attention accumulation) |
    # "no_dve" (also skip the c/h elementwise chain) | "mm_only"
    # (matmuls + DMAs only).  Non-"full" variants are timing probes.
    do_attn = variant == "full"
    do_dve = variant in ("full", "no_attn")
    do_act = variant != "mm_only"
    mdt = getattr(mybir.dt, mdt_name)
    nc = bass.Bass(target_bir_lowering=False)

    xt_d = nc.declare_dram_parameter("xt", [T, 3, 128, BS], mdt, isOutput=False)
    wc_d = nc.declare_dram_parameter("wc", [2, NK, 128, 1200], mdt, isOutput=False)
    conv_d = nc.declare_dram_parameter("convp", [128, 3], mdt, isOutput=False)
    fcw_d = nc.declare_dram_parameter("fcw", [128, 3 * NCLS], mdt, isOutput=False)
    fcb_d = nc.declare_dram_parameter("fcb", [1, NCLS], mdt, isOutput=False)
    ones_d = nc.declare_dram_parameter("onesrow", [1, BS], mdt, isOutput=False)
    out_d = nc.declare_dram_parameter("out", [BS, NCLS], f32, isOutput=True)

    with tile.TileContext(nc) as tc, ExitStack() as ctx:
        P = lambda name, bufs, **kw: ctx.enter_context(
            tc.tile_pool(name=name, bufs=bufs, **kw))
        wpool = P("w", 1)
        xpool = P("x", 3)
        zifp = P("zif", 2, space="PSUM")    # [128,1024] f32 -> 2 banks x 2
        zgop = P("zgo", 1, space="PSUM")    # [128,1024] f32 -> 2 banks
        apsp = P("aps", 1, space="PSUM")    # [1,512] -> 1 bank (reused by fc tail)
        ebpp = P("ebps", 1, space="PSUM")   # [128,512] -> 1 bank
        sifp = P("sif", 3)
        sop = P("so", 3)
        gcp = P("gc", 1)
        p1p = P("p1", 3)
        tcp = P("tc", 3)
        hp = P("h", 1)
        hsp = P("hs", 2)
        thp = P("th", 2)
        rp = P("r", 1)
        smp = P("sm", 2)
        tmpp = P("tmp", 2)
        fin = P("fin", 2)

        # ---- weights / constants ----
        wc_sb = {}
        for d in range(2):
            for k in range(NK):
                wt = wpool.tile([128, 1200], mdt, tag=f"wc_{d}_{k}")
                nc.sync.dma_start(out=wt, in_=wc_d.ap()[d, k])
                wc_sb[(d, k)] = wt
        conv_sb = wpool.tile([128, 3], mdt, tag="conv")
        nc.sync.dma_start(out=conv_sb, in_=conv_d.ap())
        fcw_sb = wpool.tile([128, 3 * NCLS], mdt, tag="fcw")
        nc.sync.dma_start(out=fcw_sb, in_=fcw_d.ap())
        fcb_sb = wpool.tile([1, NCLS], mdt, tag="fcb")
        nc.sync.dma_start(out=fcb_sb, in_=fcb_d.ap())
        ones_sb = wpool.tile([1, 128], mdt, tag="ones")
        nc.vector.memset(ones_sb, 1.0)

        # ---- persistent state ----
        # h k-tiles, parity double-buffered: step t reads kt[t%2][d],
        # writes h_t into kt[(t+1)%2][d].
        kt = {}
        for par in range(2):
            for d in range(2):
                kt[(par, d)] = []
                for j in range(3):
                    t_ = hp.tile([128, BS], mdt, tag=f"kt_{par}_{d}_{j}")
                    nc.vector.memset(t_, 0.0)
                    kt[(par, d)].append(t_)
                nc.sync.dma_start(out=kt[(par, d)][2][BIASROW:BIASROW + 1],
                                  in_=ones_d.ap())
        gc = {}    # gc[(d, j)]: [128, 1024] mdt = [tanh_g | c]
        for d in range(2):
            for j in range(3):
                g = gcp.tile([128, 1024], mdt, tag=f"gc_{d}_{j}")
                nc.vector.memset(g, 0.0)
                gc[(d, j)] = g
        r = []
        for j in range(3):
            rt = rp.tile([128, BS], f32, tag=f"r_{j}")
            nc.vector.memset(rt, 0.0)
            r.append(rt)
        ssum = rp.tile([1, BS], f32, tag="ssum")
        # timing variants skip attention: keep 1/ssum finite in the tail
        nc.vector.memset(ssum, 0.0 if do_attn else 1.0)

        def w_slice(d, k, col0, msz):
            return wc_sb[(d, k)][:, col0:col0 + msz]

        def attn_tanh(hs):
            # hs[j]: [128, BS] mdt hsum tiles from the PREVIOUS step.
            th = []
            for j in range(3):
                pmax = 45 if j == 2 else 128
                thj = thp.tile([128, BS], mdt, tag=f"th{j}")
                nc.scalar.activation(out=thj[0:pmax], in_=hs[j][0:pmax],
                                     func=AF.Tanh)
                th.append((thj, pmax))
            return th

        def attn_score(th):
            a_ps = apsp.tile([1, BS], f32, tag="a")
            for k in range(3):
                thj, pmax = th[k]
                nc.tensor.matmul(a_ps, lhsT=conv_sb[0:pmax, k:k + 1],
                                 rhs=thj[0:pmax], start=(k == 0), stop=(k == 2))
            sg = smp.tile([1, BS], f32, tag="sg")
            nc.scalar.activation(out=sg, in_=a_ps, func=AF.Sigmoid)
            om = smp.tile([1, BS], f32, tag="om")
            nc.scalar.activation(out=om, in_=sg, func=AF.Copy, bias=1.0,
                                 scale=-1.0)
            nc.vector.reciprocal(out=om, in_=om)
            e = smp.tile([1, BS], f32, tag="e")
            nc.vector.tensor_mul(out=e, in0=sg, in1=om)   # e = exp(a)
            nc.vector.tensor_add(out=ssum, in0=ssum, in1=e)
            e16 = smp.tile([1, BS], mdt, tag="e16")
            nc.scalar.activation(out=e16, in_=e, func=AF.Copy)
            return e16

        def attn_accum(hs, e16):
            eb_ps = ebpp.tile([128, BS], f32, tag="eb")
            nc.tensor.matmul(eb_ps, lhsT=ones_sb, rhs=e16, start=True, stop=True)
            for j in range(3):
                pmax = 45 if j == 2 else 128
                tmp = tmpp.tile([128, BS], f32, tag=f"tmp{j}")
                nc.vector.tensor_mul(out=tmp[0:pmax], in0=hs[j][0:pmax],
                                     in1=eb_ps[0:pmax])
                nc.gpsimd.tensor_add(out=r[j][0:pmax], in0=r[j][0:pmax],
                                     in1=tmp[0:pmax])

        def attn_tail(hs):
            attn_accum(hs, attn_score(attn_tanh(hs)))

        loop_cm = tc.For_i(0, repeat, 1) if repeat else None
        if loop_cm is not None:
            loop_cm.__enter__()

        pending_hs = None

        # ---- time loop ----
        for t in range(T):
            par, nxt = t % 2, (t + 1) % 2
            xa = xpool.tile([128, BS], mdt, tag="xa")
            nc.sync.dma_start(out=xa, in_=xt_d.ap()[t, 0])
            xb = xpool.tile([128, BS], mdt, tag="xb")
            nc.sync.dma_start(out=xb, in_=xt_d.ap()[t, 1])
            for d in range(2):
                nc.sync.dma_start(out=kt[(par, d)][2][XTAIL:XTAIL + 44],
                                  in_=xt_d.ap()[t, 2][XTAIL:XTAIL + 44])
            # previous step's attention tanh: emitted first so the th acts
            # drain ahead of this step's gate acts in the ACT FIFO.
            pend_th = attn_tanh(pending_hs) if (do_attn and pending_hs) else None
            pend_e16 = None

            hs = []
            for d in range(2):
                rhs5 = [kt[(par, d)][0], kt[(par, d)][1], kt[(par, d)][2],
                        xa, xb]
                for j, (moff, msz) in enumerate(MT):
                    sl = slice(0, msz)
                    zif = zifp.tile([128, 1024], f32, tag="zif")
                    zgo = zgop.tile([128, 1024], f32, tag="zgo")
                    for gi, zdst in ((0, zif[sl, 0:512]),
                                     (1, zif[sl, 512:1024]),
                                     (2, zgo[sl, 0:512]),
                                     (3, zgo[sl, 512:1024])):
                        col0 = GOFF[gi] + moff
                        for k in range(NK):
                            nc.tensor.matmul(
                                zdst, lhsT=w_slice(d, k, col0, msz),
                                rhs=rhs5[k], start=(k == 0), stop=(k == NK - 1))
                    if not do_act:
                        continue
                    sif = sifp.tile([128, 1024], mdt, tag="sif")
                    nc.scalar.activation(out=sif[sl], in_=zif[sl],
                                         func=AF.Sigmoid)
                    gcj = gc[(d, j)]
                    nc.scalar.activation(out=gcj[sl, 0:512], in_=zgo[sl, 0:512],
                                         func=AF.Tanh)
                    so = sop.tile([128, BS], mdt, tag="so")
                    nc.scalar.activation(out=so[sl], in_=zgo[sl, 512:1024],
                                         func=AF.Sigmoid)
                    if not do_dve:
                        continue
                    # c_new = sig_f * c + sig_i * tanh_g ; h = sig_o * tanh(c)
                    p1 = p1p.tile([128, 1024], mdt, tag="p1")
                    nc.vector.tensor_mul(out=p1[sl], in0=sif[sl], in1=gcj[sl])
                    nc.vector.tensor_add(out=gcj[sl, 512:1024],
                                         in0=p1[sl, 0:512], in1=p1[sl, 512:1024])
                    tcj = tcp.tile([128, BS], mdt, tag="tc")
                    nc.scalar.activation(out=tcj[sl], in_=gcj[sl, 512:1024],
                                         func=AF.Tanh)
                    # h_t lands directly in the next step's rhs k-tile
                    nc.vector.tensor_mul(out=kt[(nxt, d)][j][sl], in0=so[sl],
                                         in1=tcj[sl])
                    if d == 1 and do_attn:
                        pmax = 45 if j == 2 else 128
                        hsj = hsp.tile([128, BS], mdt, tag=f"hs{j}")
                        nc.vector.tensor_add(out=hsj[0:pmax],
                                             in0=kt[(nxt, 0)][j][0:pmax],
                                             in1=kt[(nxt, 1)][j][0:pmax])
                        hs.append(hsj)
                    # previous step's attention, staged so its PE ops never
                    # wait on its ACT/DVE chain: score after d0-j0 (conv
                    # matmuls see ready th, e16 chain overlaps d0-j1/j2),
                    # accumulate after d1-j0 (eb matmul sees ready e16).
                    if pend_th is not None:
                        if d == 0 and j == 0:
                            pend_e16 = attn_score(pend_th)
                        elif d == 1 and j == 0:
                            attn_accum(pending_hs, pend_e16)
            pending_hs = hs

        if do_attn:
            attn_tail(pending_hs)

        if loop_cm is not None:
            loop_cm.__exit__(None, None, None)

        # ---- tail: hStar = tanh(r / s); logits; softmax ----
        rs = smp.tile([1, BS], f32, tag="rs")
        nc.vector.reciprocal(out=rs, in_=ssum)
        rs16 = smp.tile([1, BS], mdt, tag="rs16")
        nc.scalar.activation(out=rs16, in_=rs, func=AF.Copy)
        rsb = ebpp.tile([128, BS], f32, tag="eb")
        nc.tensor.matmul(rsb, lhsT=ones_sb, rhs=rs16, start=True, stop=True)
        hst = []
        for j in range(3):
            hn = fin.tile([128, BS], f32, tag=f"hn{j}")
            nc.vector.tensor_mul(out=hn, in0=r[j], in1=rsb)
            hj = fin.tile([128, BS], mdt, tag=f"hst{j}")
            nc.scalar.activation(out=hj, in_=hn, func=AF.Tanh)
            hst.append(hj)
        for bt in range(BS // 128):
            fcp = apsp.tile([128, NCLS], f32, tag="a")
            for j in range(3):
                nc.tensor.matmul(fcp, lhsT=hst[j][:, bt * 128:(bt + 1) * 128],
                                 rhs=fcw_sb[:, j * NCLS:(j + 1) * NCLS],
                                 start=(j == 0), stop=False)
            nc.tensor.matmul(fcp, lhsT=ones_sb, rhs=fcb_sb, start=False, stop=True)
            mx = fin.tile([128, 1], f32, tag="mx")
            nc.vector.reduce_max(out=mx, in_=fcp, axis=AX.X)
            nmx = fin.tile([128, 1], f32, tag="nmx")
            nc.vector.tensor_scalar_mul(out=nmx, in0=mx, scalar1=-1.0)
            ex = fin.tile([128, NCLS], f32, tag="ex")
            nc.scalar.activation(out=ex, in_=fcp, func=AF.Exp, bias=nmx)
            sm = fin.tile([128, 1], f32, tag="smm")
            nc.vector.reduce_sum(out=sm, in_=ex, axis=AX.X)
            nc.vector.reciprocal(out=sm, in_=sm)
            ot = fin.tile([128, NCLS], f32, tag="ot")
            nc.vector.tensor_scalar_mul(out=ot, in0=ex, scalar1=sm)
            nc.sync.dma_start(out=out_d.ap()[bt * 128:(bt + 1) * 128], in_=ot)

    return nc


def _prep(x, w_ih, w_hh, b_ih, b_hh, conv_w, fc_w, fc_b, np_mdt):
    """Host-side layout prep (shared across cores + per-core x shards).

    Merged contraction rows (640 = 5 k-tiles of 128):
      tile 0: h[0:128]        tile 1: h[128:256]
      tile 2: h[256:300] at parts 0..43, bias (const-1 row) at part 44,
              x[256:300] at parts 64..107, zeros elsewhere
      tile 3: x[0:128]        tile 4: x[128:256]
    """
    bias = (b_ih + b_hh).astype(np.float32)  # [2, 1200]
    wc = np.zeros((2, NK, 128, 1200), np.float32)
    for d in range(2):
        comb = np.zeros((NK * 128, 1200), np.float32)
        comb[0:256] = w_hh[d].T[0:256]
        comb[256:300] = w_hh[d].T[256:300]
        comb[256 + BIASROW] = bias[d]
        comb[256 + XTAIL:256 + XTAIL + 44] = w_ih[d].T[256:300]
        comb[384:512] = w_ih[d].T[0:128]
        comb[512:640] = w_ih[d].T[128:256]
        wc[d] = comb.reshape(NK, 128, 1200)

    def h_pack(vec_or_mat, width):
        """Pack [300(, width)] h-feature data into the 3-tile h k-layout."""
        out = np.zeros((3, 128, width), np.float32)
        v = vec_or_mat.reshape(H, width)
        out[0] = v[0:128]
        out[1] = v[128:256]
        out[2, 0:44] = v[256:300]
        return out

    convp = np.ascontiguousarray(
        h_pack(conv_w, 1).reshape(3, 128).T)          # [128, 3]
    fcw = np.ascontiguousarray(
        h_pack(fc_w.T, NCLS).transpose(1, 0, 2).reshape(128, 3 * NCLS))

    shared = {
        "wc": wc.astype(np_mdt),
        "convp": convp.astype(np_mdt),
        "fcw": fcw.astype(np_mdt),
        "fcb": fc_b.reshape(1, NCLS).astype(np_mdt),
        "onesrow": np.ones((1, BS), np.float32).astype(np_mdt),
    }

    # x: [B, H, T] -> per-core [T, 3, 128, BS]:
    # slot 0 = x[0:128], slot 1 = x[128:256],
    # slot 2 = zeros with x[256:300] at parts 64..107.
    xs = np.ascontiguousarray(np.transpose(x, (2, 1, 0)))  # [T, H, B]
    xp = np.zeros((T, 3, 128, B), np.float32)
    xp[:, 0] = xs[:, 0:128]
    xp[:, 1] = xs[:, 128:256]
    xp[:, 2, XTAIL:XTAIL + 44] = xs[:, 256:300]
    xp = xp.reshape(T, 3, 128, NCORES, BS)
    in_maps = []
    for c in range(NCORES):
        m = dict(shared)
        m["xt"] = np.ascontiguousarray(xp[:, :, :, c]).astype(np_mdt)
        in_maps.append(m)
    return in_maps


def _np_mdt(mdt_name):
    return np.float16 if mdt_name == "float16" else (
        __import__("ml_dtypes").bfloat16 if mdt_name == "bfloat16" else np.float32)


def _runner(repeat=0, variant="full"):
    key = (MM_DT_NAME, repeat, variant)
    if key not in _CACHE:
        _CACHE[key] = _Runner(_build(MM_DT_NAME, repeat=repeat,
                                     variant=variant), NCORES)
    return _CACHE[key]


def _in_maps(inputs_f32):
    return _prep(*inputs_f32, _np_mdt(MM_DT_NAME))


def _inputs_f32(x, w_ih, w_hh, b_ih, b_hh, conv_w, fc_w, fc_b):
    return [np.asarray(a, np.float32) for a in
            (x, w_ih, w_hh, b_ih, b_hh, conv_w, fc_w, fc_b)]


def kernel(x, w_ih, w_hh, b_ih, b_hh, conv_w, fc_w, fc_b):
    runner = _runner(repeat=0)
    in_maps = _in_maps(_inputs_f32(x, w_ih, w_hh, b_ih, b_hh,
                                   conv_w, fc_w, fc_b))
    results = runner.run(in_maps)
    out = np.concatenate([r["out"] for r in results], axis=0)
    return out.astype(np.float32)


def bench(x, w_ih, w_hh, b_ih, b_hh, conv_w, fc_w, fc_b, iters=5):
    runner = _runner(repeat=0)
    in_maps = _in_maps(_inputs_f32(x, w_ih, w_hh, b_ih, b_hh,
                                   conv_w, fc_w, fc_b))
    return runner.bench(in_maps, iters=iters)


def measure_exec_ns(inputs, r_lo=1, r_hi=301, iters=10):
    """Device execution time of one full forward pass, in ns.

    The axon tunnel adds a fixed ~70-80 ms completion-notification latency
    to every blocking call, independent of what the NEFF does (measured:
    a trivial 4-instruction kernel takes the same wall time as the full
    LSTM).  To measure hardware execution, both builds wrap the whole
    T-step forward in a hardware For_i loop (r_lo vs r_hi iterations,
    identical instruction stream per iteration); the slope
    (min_wall(r_hi) - min_wall(r_lo)) / (r_hi - r_lo) is the steady-state
    on-device time of one forward pass with the constant latency cancelled.
    Samples are interleaved so network drift affects both arms equally.
    """
    import time
    in_maps = _in_maps(_inputs_f32(**inputs) if isinstance(inputs, dict)
                       else _inputs_f32(*inputs))
    runners = {rep: _runner(repeat=rep) for rep in (r_lo, r_hi)}
    dev_in = {rep: runners[rep].put_inputs(in_maps) for rep in (r_lo, r_hi)}
    for rep in (r_lo, r_hi):
        runners[rep].call(dev_in[rep])  # warm
    walls = {r_lo: [], r_hi: []}
    for _ in range(iters):
        for rep in (r_lo, r_hi):
            t0 = time.perf_counter()
            runners[rep].call(dev_in[rep])
            walls[rep].append(time.perf_counter() - t0)
    lo, hi = min(walls[r_lo]), min(walls[r_hi])
    ns = (hi - lo) * 1e9 / (r_hi - r_lo)
    return max(int(ns), 1), walls


# revision 15
# speedup vs baseline: 73.1398x; 1.0719x over previous
"""AttentionLSTM Trainium2 kernel — 8-core data-parallel.

Model (per batch row b): two independent single-direction LSTMs over T=43
steps of x[:, :, t] (H=300 features), hidden states summed, then a
conv-softmax attention over time, tanh, fc(300->80), softmax.

Device mapping per core (512 batch rows):
  - z^T[1200, 512] per (direction, step) via PE matmuls: merged contraction
    K=5 k-tiles of 128 (h rows 0..299 first, then bias + x tail, then x),
    M gate-aligned tiles {128,128,44}, fp16 MM inputs, fp32 PSUM accum.
  - h k-tiles are parity double-buffered: step t reads kt[t%2], writes h_t
    into kt[(t+1)%2], so every matmul of a step sees the full h_{t-1}
    (exact LSTM semantics, no Gauss-Seidel staleness).
  - group order is d-major (all of direction 0, then direction 1): d0's
    elementwise tail hides under d1's matmuls and vice versa across the
    step boundary, keeping the PE dense.
  - gates: one fused sigmoid over an [*,1024] i|f PSUM pair, tanh(g) and
    sigmoid(o) over a shared g|o PSUM pair; gate/cell elementwise state in
    fp16 for 2x DVE throughput; c stays in SBUF.
  - attention accumulated online: e_t = sigmoid(a)/(1-sigmoid(a)) = exp(a)
    (avoids exp table loads mid-loop); e_t broadcast across partitions with
    a rank-1 PE matmul (ones x e) into PSUM — no DRAM round trip; r += on
    GPSIMD.
  - tail: hStar = tanh(r/s), logits = fc(hStar) via PE (batch on PSUM
    partitions), softmax over the 80-class free dim.
"""

import os
import sys

sys.path.insert(0, "/opt/trn_rl_repo")

from contextlib import ExitStack

import numpy as np

import concourse.bass as bass
import concourse.tile as tile
from concourse import mybir
from concourse.bass_utils import run_bass_kernel_spmd  # noqa: F401  (spmd path kept available)

f32 = mybir.dt.float32
AF = mybir.ActivationFunctionType
AX = mybir.AxisListType

_BIRFIX_DONE = False


def _split_multiwaits(bir_json):
    """This walrus build allows one sync-wait per engine instruction; Tile
    attaches one per producer proc. Hoist extras onto standalone
    EventSemaphore instructions inserted just before, same engine queue."""
    import json
    j = json.loads(bir_json.decode() if isinstance(bir_json, bytes) else bir_json)
    for fn in j.get("functions", []):
        for blk in fn.get("blocks", []):
            out = []
            for ins in blk.get("instructions", []):
                si = ins.get("sync_info")
                ow = si.get("on_wait") if si else None
                if ow and len(ow) > 1:
                    for i, w in enumerate(ow[:-1]):
                        out.append({
                            "debug": ins.get("debug", 0),
                            "engine": ins["engine"],
                            "ins": [], "outs": [],
                            "name": f"{ins['name']}_xw{i}",
                            "opcode": "EventSemaphore",
                            "sync_info": {"on_update": [], "on_wait": [w]},
                        })
                    si["on_wait"] = [ow[-1]]
                out.append(ins)
            blk["instructions"] = out
    return json.dumps(j).encode()


def _install_birfix():
    global _BIRFIX_DONE
    if _BIRFIX_DONE:
        return
    from concourse import bass2jax
    orig = bass2jax.compile_bir_kernel

    def patched(bir_json, tmpdir, neff_name="file.neff"):
        return orig(_split_multiwaits(bir_json), tmpdir, neff_name)

    bass2jax.compile_bir_kernel = patched
    _BIRFIX_DONE = True


class _Runner:
    """Compile once; keep the sharded jitted executable + device inputs."""

    def __init__(self, nc, n_cores):
        import jax
        from jax.sharding import Mesh, PartitionSpec
        from jax.experimental.shard_map import shard_map
        from concourse import bass2jax as b2j

        b2j.install_neuronx_cc_hook()
        _install_birfix()
        self.jax = jax
        self.nc = nc
        self.n_cores = n_cores
        part_name = nc.partition_id_tensor.name if nc.partition_id_tensor else None
        in_names, out_names, out_avals, zero_outs = [], [], [], []
        for alloc in nc.m.functions[0].allocations:
            if not isinstance(alloc, mybir.MemoryLocationSet):
                continue
            name = alloc.memorylocations[0].name
            if alloc.kind == "ExternalInput":
                if name != part_name:
                    in_names.append(name)
            elif alloc.kind == "ExternalOutput":
                out_names.append(name)
                shape = tuple(alloc.tensor_shape)
                dtype = mybir.dt.np(alloc.dtype)
                out_avals.append(jax.core.ShapedArray(shape, dtype))
                zero_outs.append(np.zeros(shape, dtype))
        self.in_names = list(in_names)
        self.out_names = out_names
        self.out_avals = out_avals
        self.zero_outs = zero_outs
        n_params = len(in_names)
        n_outs = len(out_avals)
        all_names = in_names + out_names
        if part_name is not None:
            all_names = all_names + [part_name]
        donate = tuple(range(n_params, n_params + n_outs))

        def _body(*args):
            operands = list(args)
            if part_name is not None:
                operands.append(b2j.partition_id_tensor())
            outs = b2j._bass_exec_p.bind(
                *operands,
                out_avals=tuple(out_avals),
                in_names=tuple(all_names),
                out_names=tuple(out_names),
                lowering_input_output_aliases=(),
                sim_require_finite=True,
                sim_require_nnan=True,
                nc=nc,
            )
            return tuple(outs)

        devices = jax.devices()[:n_cores]
        self.mesh = Mesh(np.asarray(devices), ("core",))
        in_specs = (PartitionSpec("core"),) * (n_params + n_outs)
        out_specs = (PartitionSpec("core"),) * n_outs
        self.sharded = jax.jit(
            shard_map(_body, mesh=self.mesh, in_specs=in_specs,
                      out_specs=out_specs, check_rep=False),
            donate_argnums=donate, keep_unused=True)
        self.sharding = jax.sharding.NamedSharding(
            self.mesh, PartitionSpec("core"))

    def put_inputs(self, in_maps):
        jax = self.jax
        concat = [np.concatenate([np.asarray(m[n]) for m in in_maps], axis=0)
                  for n in self.in_names]
        return [jax.device_put(a, self.sharding) for a in concat]

    def call(self, dev_in):
        zeros = [np.zeros((self.n_cores * z.shape[0], *z.shape[1:]), z.dtype)
                 for z in self.zero_outs]
        outs = self.sharded(*dev_in, *zeros)
        self.jax.block_until_ready(outs)
        return outs

    def run(self, in_maps):
        dev_in = self.put_inputs(in_maps)
        outs = self.call(dev_in)
        n = self.n_cores
        return [
            {name: np.asarray(outs[i]).reshape(n, *self.out_avals[i].shape)[c]
             for i, name in enumerate(self.out_names)}
            for c in range(n)
        ]

    def bench(self, in_maps, iters=5):
        import time
        dev_in = self.put_inputs(in_maps)
        self.call(dev_in)  # warm
        times = []
        for _ in range(iters):
            t0 = time.perf_counter()
            self.call(dev_in)
            times.append(time.perf_counter() - t0)
        return times


B, H, T, NCLS = 4096, 300, 43, 80
NCORES = 8
BS = B // NCORES          # 512 batch rows per core
NK = 5                    # k-tiles: [h0:128 | h128:256 | h256:300+bias+xtail | x0:128 | x128:256]
BIASROW = 44              # partition of the bias (constant-1) row in k-tile 2
XTAIL = 64                # x rows 256..300 live at parts 64..108 of k-tile 2
MT = [(0, 128), (128, 128), (256, 44)]    # (moff, msz) per gate, output base partition 0
GOFF = [0, 300, 600, 900]                 # torch gate order i,f,g,o

MM_DT_NAME = os.environ.get("LSTM_MM_DT", "float16")

_CACHE = {}


def _build(mdt_name, repeat=0, variant="full"):
    # variant: "full" | "no_attn" (skip attention accumulation) |
    # "no_dve" (also skip the c/h elementwise chain) | "mm_only"
    # (matmuls + DMAs only) | "mm_nodma" (matmuls, static rhs) |
    # "mm_n256" (matmuls at N=256).  Non-"full" variants are timing probes.
    do_attn = variant == "full"
    do_dve = variant in ("full", "no_attn")
    do_act = variant not in ("mm_only", "mm_nodma", "mm_n256")
    do_xdma = variant != "mm_nodma"
    ncols = 256 if variant == "mm_n256" else 512
    mdt = getattr(mybir.dt, mdt_name)
    nc = bass.Bass(target_bir_lowering=False)

    xt_d = nc.declare_dram_parameter("xt", [T, 3, 128, BS], mdt, isOutput=False)
    wc_d = nc.declare_dram_parameter("wc", [2, NK, 128, 1200], mdt, isOutput=False)
    conv_d = nc.declare_dram_parameter("convp", [128, 3], mdt, isOutput=False)
    fcw_d = nc.declare_dram_parameter("fcw", [128, 3 * NCLS], mdt, isOutput=False)
    fcb_d = nc.declare_dram_parameter("fcb", [1, NCLS], mdt, isOutput=False)
    ones_d = nc.declare_dram_parameter("onesrow", [1, BS], mdt, isOutput=False)
    out_d = nc.declare_dram_parameter("out", [BS, NCLS], f32, isOutput=True)

    with tile.TileContext(nc) as tc, ExitStack() as ctx:
        P = lambda name, bufs, **kw: ctx.enter_context(
            tc.tile_pool(name=name, bufs=bufs, **kw))
        wpool = P("w", 1)
        xpool = P("x", 3)
        # One shared pool for all gate PSUM tiles: 3 x [128,1024] f32 =
        # 6 banks.  With separate zif(bufs=2)/zgo(bufs=1) pools the g|o
        # matmuls of each group waited on the previous group's o-act drain
        # with ~0 margin -> ~1us PE stall per group (~260us/forward).
        zp = P("z", 3, space="PSUM")
        # Attention score [1,512] and broadcast [128,512] share one slot
        # tag (strictly sequential within a step); 2 bufs = 2 banks.
        atp = P("at", 2, space="PSUM")
        sifp = P("sif", 3)
        sop = P("so", 3)
        gcp = P("gc", 1)
        p1p = P("p1", 3)
        tcp = P("tc", 3)
        hp = P("h", 1)
        hsp = P("hs", 2)
        thp = P("th", 2)
        rp = P("r", 1)
        smp = P("sm", 2)
        tmpp = P("tmp", 2)
        fin = P("fin", 2)

        # ---- weights / constants ----
        wc_sb = {}
        for d in range(2):
            for k in range(NK):
                wt = wpool.tile([128, 1200], mdt, tag=f"wc_{d}_{k}")
                nc.sync.dma_start(out=wt, in_=wc_d.ap()[d, k])
                wc_sb[(d, k)] = wt
        conv_sb = wpool.tile([128, 3], mdt, tag="conv")
        nc.sync.dma_start(out=conv_sb, in_=conv_d.ap())
        fcw_sb = wpool.tile([128, 3 * NCLS], mdt, tag="fcw")
        nc.sync.dma_start(out=fcw_sb, in_=fcw_d.ap())
        fcb_sb = wpool.tile([1, NCLS], mdt, tag="fcb")
        nc.sync.dma_start(out=fcb_sb, in_=fcb_d.ap())
        ones_sb = wpool.tile([1, 128], mdt, tag="ones")
        nc.vector.memset(ones_sb, 1.0)

        # ---- persistent state ----
        # h k-tiles, parity double-buffered: step t reads kt[t%2][d],
        # writes h_t into kt[(t+1)%2][d].
        kt = {}
        for par in range(2):
            for d in range(2):
                kt[(par, d)] = []
                for j in range(3):
                    t_ = hp.tile([128, BS], mdt, tag=f"kt_{par}_{d}_{j}")
                    nc.vector.memset(t_, 0.0)
                    kt[(par, d)].append(t_)
                nc.sync.dma_start(out=kt[(par, d)][2][BIASROW:BIASROW + 1],
                                  in_=ones_d.ap())
        gc = {}    # gc[(d, j)]: [128, 1024] mdt = [tanh_g | c]
        for d in range(2):
            for j in range(3):
                g = gcp.tile([128, 1024], mdt, tag=f"gc_{d}_{j}")
                nc.vector.memset(g, 0.0)
                gc[(d, j)] = g
        r = []
        for j in range(3):
            rt = rp.tile([128, BS], f32, tag=f"r_{j}")
            nc.vector.memset(rt, 0.0)
            r.append(rt)
        ssum = rp.tile([1, BS], f32, tag="ssum")
        # timing variants skip attention: keep 1/ssum finite in the tail
        nc.vector.memset(ssum, 0.0 if do_attn else 1.0)

        def w_slice(d, k, col0, msz):
            return wc_sb[(d, k)][:, col0:col0 + msz]

        def attn_tanh(hs):
            # hs[j]: [128, BS] mdt hsum tiles from the PREVIOUS step.
            th = []
            for j in range(3):
                pmax = 45 if j == 2 else 128
                thj = thp.tile([128, BS], mdt, tag=f"th{j}")
                nc.scalar.activation(out=thj[0:pmax], in_=hs[j][0:pmax],
                                     func=AF.Tanh)
                th.append((thj, pmax))
            return th

        def attn_score(th):
            a_ps = atp.tile([1, BS], f32, tag="at")
            for k in range(3):
                thj, pmax = th[k]
                nc.tensor.matmul(a_ps, lhsT=conv_sb[0:pmax, k:k + 1],
                                 rhs=thj[0:pmax], start=(k == 0), stop=(k == 2))
            sg = smp.tile([1, BS], f32, tag="sg")
            nc.scalar.activation(out=sg, in_=a_ps, func=AF.Sigmoid)
            om = smp.tile([1, BS], f32, tag="om")
            nc.scalar.activation(out=om, in_=sg, func=AF.Copy, bias=1.0,
                                 scale=-1.0)
            nc.vector.reciprocal(out=om, in_=om)
            e = smp.tile([1, BS], f32, tag="e")
            nc.vector.tensor_mul(out=e, in0=sg, in1=om)   # e = exp(a)
            nc.vector.tensor_add(out=ssum, in0=ssum, in1=e)
            e16 = smp.tile([1, BS], mdt, tag="e16")
            nc.scalar.activation(out=e16, in_=e, func=AF.Copy)
            return e16

        def attn_accum(hs, e16):
            eb_ps = atp.tile([128, BS], f32, tag="at")
            nc.tensor.matmul(eb_ps, lhsT=ones_sb, rhs=e16, start=True, stop=True)
            for j in range(3):
                pmax = 45 if j == 2 else 128
                tmp = tmpp.tile([128, BS], f32, tag=f"tmp{j}")
                nc.vector.tensor_mul(out=tmp[0:pmax], in0=hs[j][0:pmax],
                                     in1=eb_ps[0:pmax])
                nc.gpsimd.tensor_add(out=r[j][0:pmax], in0=r[j][0:pmax],
                                     in1=tmp[0:pmax])

        def attn_tail(hs):
            attn_accum(hs, attn_score(attn_tanh(hs)))

        loop_cm = tc.For_i(0, repeat, 1) if repeat else None
        if loop_cm is not None:
            loop_cm.__enter__()

        pending_hs = None
        if not do_xdma:
            xa0 = xpool.tile([128, BS], mdt, tag="xa")
            nc.vector.memset(xa0, 0.0)
            xb0 = xpool.tile([128, BS], mdt, tag="xb")
            nc.vector.memset(xb0, 0.0)

        # ---- time loop ----
        for t in range(T):
            par, nxt = t % 2, (t + 1) % 2
            if do_xdma:
                xa = xpool.tile([128, BS], mdt, tag="xa")
                nc.sync.dma_start(out=xa, in_=xt_d.ap()[t, 0])
                xb = xpool.tile([128, BS], mdt, tag="xb")
                nc.sync.dma_start(out=xb, in_=xt_d.ap()[t, 1])
                for d in range(2):
                    nc.sync.dma_start(out=kt[(par, d)][2][XTAIL:XTAIL + 44],
                                      in_=xt_d.ap()[t, 2][XTAIL:XTAIL + 44])
            else:
                xa, xb = xa0, xb0
            # previous step's attention tanh: emitted first so the th acts
            # drain ahead of this step's gate acts in the ACT FIFO.
            pend_th = attn_tanh(pending_hs) if (do_attn and pending_hs) else None
            pend_e16 = None

            hs = []
            for d in range(2):
                rhs5 = [kt[(par, d)][0], kt[(par, d)][1], kt[(par, d)][2],
                        xa, xb]
                for j, (moff, msz) in enumerate(MT):
                    sl = slice(0, msz)
                    zif = zp.tile([128, 1024], f32, tag="z")
                    zgo = zp.tile([128, 1024], f32, tag="z")
                    for gi, zdst in ((0, zif[sl, 0:ncols]),
                                     (1, zif[sl, 512:512 + ncols]),
                                     (2, zgo[sl, 0:ncols]),
                                     (3, zgo[sl, 512:512 + ncols])):
                        col0 = GOFF[gi] + moff
                        for k in range(NK):
                            nc.tensor.matmul(
                                zdst, lhsT=w_slice(d, k, col0, msz),
                                rhs=rhs5[k][:, 0:ncols],
                                start=(k == 0), stop=(k == NK - 1))
                    if not do_act:
                        continue
                    sif = sifp.tile([128, 1024], mdt, tag="sif")
                    nc.scalar.activation(out=sif[sl], in_=zif[sl],
                                         func=AF.Sigmoid)
                    gcj = gc[(d, j)]
                    nc.scalar.activation(out=gcj[sl, 0:512], in_=zgo[sl, 0:512],
                                         func=AF.Tanh)
                    so = sop.tile([128, BS], mdt, tag="so")
                    nc.scalar.activation(out=so[sl], in_=zgo[sl, 512:1024],
                                         func=AF.Sigmoid)
                    if not do_dve:
                        continue
                    # c_new = sig_f * c + sig_i * tanh_g ; h = sig_o * tanh(c)
                    p1 = p1p.tile([128, 1024], mdt, tag="p1")
                    nc.vector.tensor_mul(out=p1[sl], in0=sif[sl], in1=gcj[sl])
                    nc.vector.tensor_add(out=gcj[sl, 512:1024],
                                         in0=p1[sl, 0:512], in1=p1[sl, 512:1024])
                    tcj = tcp.tile([128, BS], mdt, tag="tc")
                    nc.scalar.activation(out=tcj[sl], in_=gcj[sl, 512:1024],
                                         func=AF.Tanh)
                    # h_t lands directly in the next step's rhs k-tile
                    nc.vector.tensor_mul(out=kt[(nxt, d)][j][sl], in0=so[sl],
                                         in1=tcj[sl])
                    if d == 1 and do_attn:
                        pmax = 45 if j == 2 else 128
                        hsj = hsp.tile([128, BS], mdt, tag=f"hs{j}")
                        nc.vector.tensor_add(out=hsj[0:pmax],
                                             in0=kt[(nxt, 0)][j][0:pmax],
                                             in1=kt[(nxt, 1)][j][0:pmax])
                        hs.append(hsj)
                    # previous step's attention, staged so its PE ops never
                    # wait on its ACT/DVE chain: score after d0-j0 (conv
                    # matmuls see ready th, e16 chain overlaps d0-j1/j2),
                    # accumulate after d1-j0 (eb matmul sees ready e16).
                    if pend_th is not None:
                        if d == 0 and j == 1:
                            pend_e16 = attn_score(pend_th)
                        elif d == 1 and j == 0:
                            attn_accum(pending_hs, pend_e16)
            pending_hs = hs

        if do_attn:
            attn_tail(pending_hs)

        if loop_cm is not None:
            loop_cm.__exit__(None, None, None)

        # ---- tail: hStar = tanh(r / s); logits; softmax ----
        rs = smp.tile([1, BS], f32, tag="rs")
        nc.vector.reciprocal(out=rs, in_=ssum)
        rs16 = smp.tile([1, BS], mdt, tag="rs16")
        nc.scalar.activation(out=rs16, in_=rs, func=AF.Copy)
        rsb = atp.tile([128, BS], f32, tag="at")
        nc.tensor.matmul(rsb, lhsT=ones_sb, rhs=rs16, start=True, stop=True)
        hst = []
        for j in range(3):
            hn = fin.tile([128, BS], f32, tag=f"hn{j}")
            nc.vector.tensor_mul(out=hn, in0=r[j], in1=rsb)
            hj = fin.tile([128, BS], mdt, tag=f"hst{j}")
            nc.scalar.activation(out=hj, in_=hn, func=AF.Tanh)
            hst.append(hj)
        for bt in range(BS // 128):
            fcp = atp.tile([128, NCLS], f32, tag="at")
            for j in range(3):
                nc.tensor.matmul(fcp, lhsT=hst[j][:, bt * 128:(bt + 1) * 128],
                                 rhs=fcw_sb[:, j * NCLS:(j + 1) * NCLS],
                                 start=(j == 0), stop=False)
            nc.tensor.matmul(fcp, lhsT=ones_sb, rhs=fcb_sb, start=False, stop=True)
            mx = fin.tile([128, 1], f32, tag="mx")
            nc.vector.reduce_max(out=mx, in_=fcp, axis=AX.X)
            nmx = fin.tile([128, 1], f32, tag="nmx")
            nc.vector.tensor_scalar_mul(out=nmx, in0=mx, scalar1=-1.0)
            ex = fin.tile([128, NCLS], f32, tag="ex")
            nc.scalar.activation(out=ex, in_=fcp, func=AF.Exp, bias=nmx)
            sm = fin.tile([128, 1], f32, tag="smm")
            nc.vector.reduce_sum(out=sm, in_=ex, axis=AX.X)
            nc.vector.reciprocal(out=sm, in_=sm)
            ot = fin.tile([128, NCLS], f32, tag="ot")
            nc.vector.tensor_scalar_mul(out=ot, in0=ex, scalar1=sm)
            nc.sync.dma_start(out=out_d.ap()[bt * 128:(bt + 1) * 128], in_=ot)

    return nc


def _prep(x, w_ih, w_hh, b_ih, b_hh, conv_w, fc_w, fc_b, np_mdt):
    """Host-side layout prep (shared across cores + per-core x shards).

    Merged contraction rows (640 = 5 k-tiles of 128):
      tile 0: h[0:128]        tile 1: h[128:256]
      tile 2: h[256:300] at parts 0..43, bias (const-1 row) at part 44,
              x[256:300] at parts 64..107, zeros elsewhere
      tile 3: x[0:128]        tile 4: x[128:256]
    """
    bias = (b_ih + b_hh).astype(np.float32)  # [2, 1200]
    wc = np.zeros((2, NK, 128, 1200), np.float32)
    for d in range(2):
        comb = np.zeros((NK * 128, 1200), np.float32)
        comb[0:256] = w_hh[d].T[0:256]
        comb[256:300] = w_hh[d].T[256:300]
        comb[256 + BIASROW] = bias[d]
        comb[256 + XTAIL:256 + XTAIL + 44] = w_ih[d].T[256:300]
        comb[384:512] = w_ih[d].T[0:128]
        comb[512:640] = w_ih[d].T[128:256]
        wc[d] = comb.reshape(NK, 128, 1200)

    def h_pack(vec_or_mat, width):
        """Pack [300(, width)] h-feature data into the 3-tile h k-layout."""
        out = np.zeros((3, 128, width), np.float32)
        v = vec_or_mat.reshape(H, width)
        out[0] = v[0:128]
        out[1] = v[128:256]
        out[2, 0:44] = v[256:300]
        return out

    convp = np.ascontiguousarray(
        h_pack(conv_w, 1).reshape(3, 128).T)          # [128, 3]
    fcw = np.ascontiguousarray(
        h_pack(fc_w.T, NCLS).transpose(1, 0, 2).reshape(128, 3 * NCLS))

    shared = {
        "wc": wc.astype(np_mdt),
        "convp": convp.astype(np_mdt),
        "fcw": fcw.astype(np_mdt),
        "fcb": fc_b.reshape(1, NCLS).astype(np_mdt),
        "onesrow": np.ones((1, BS), np.float32).astype(np_mdt),
    }

    # x: [B, H, T] -> per-core [T, 3, 128, BS]:
    # slot 0 = x[0:128], slot 1 = x[128:256],
    # slot 2 = zeros with x[256:300] at parts 64..107.
    xs = np.ascontiguousarray(np.transpose(x, (2, 1, 0)))  # [T, H, B]
    xp = np.zeros((T, 3, 128, B), np.float32)
    xp[:, 0] = xs[:, 0:128]
    xp[:, 1] = xs[:, 128:256]
    xp[:, 2, XTAIL:XTAIL + 44] = xs[:, 256:300]
    xp = xp.reshape(T, 3, 128, NCORES, BS)
    in_maps = []
    for c in range(NCORES):
        m = dict(shared)
        m["xt"] = np.ascontiguousarray(xp[:, :, :, c]).astype(np_mdt)
        in_maps.append(m)
    return in_maps


def _np_mdt(mdt_name):
    return np.float16 if mdt_name == "float16" else (
        __import__("ml_dtypes").bfloat16 if mdt_name == "bfloat16" else np.float32)


def _runner(repeat=0, variant="full"):
    key = (MM_DT_NAME, repeat, variant)
    if key not in _CACHE:
        _CACHE[key] = _Runner(_build(MM_DT_NAME, repeat=repeat,
                                     variant=variant), NCORES)
    return _CACHE[key]


def _in_maps(inputs_f32):
    return _prep(*inputs_f32, _np_mdt(MM_DT_NAME))


def _inputs_f32(x, w_ih, w_hh, b_ih, b_hh, conv_w, fc_w, fc_b):
    return [np.asarray(a, np.float32) for a in
            (x, w_ih, w_hh, b_ih, b_hh, conv_w, fc_w, fc_b)]


def kernel(x, w_ih, w_hh, b_ih, b_hh, conv_w, fc_w, fc_b):
    runner = _runner(repeat=0)
    in_maps = _in_maps(_inputs_f32(x, w_ih, w_hh, b_ih, b_hh,
                                   conv_w, fc_w, fc_b))
    results = runner.run(in_maps)
    out = np.concatenate([r["out"] for r in results], axis=0)
    return out.astype(np.float32)


def bench(x, w_ih, w_hh, b_ih, b_hh, conv_w, fc_w, fc_b, iters=5):
    runner = _runner(repeat=0)
    in_maps = _in_maps(_inputs_f32(x, w_ih, w_hh, b_ih, b_hh,
                                   conv_w, fc_w, fc_b))
    return runner.bench(in_maps, iters=iters)


def measure_exec_ns(inputs, r_lo=1, r_hi=301, iters=10):
    """Device execution time of one full forward pass, in ns.

    The axon tunnel adds a fixed ~70-80 ms completion-notification latency
    to every blocking call, independent of what the NEFF does (measured:
    a trivial 4-instruction kernel takes the same wall time as the full
    LSTM).  To measure hardware execution, both builds wrap the whole
    T-step forward in a hardware For_i loop (r_lo vs r_hi iterations,
    identical instruction stream per iteration); the slope
    (min_wall(r_hi) - min_wall(r_lo)) / (r_hi - r_lo) is the steady-state
    on-device time of one forward pass with the constant latency cancelled.
    Samples are interleaved so network drift affects both arms equally.
    """
    import time
    in_maps = _in_maps(_inputs_f32(**inputs) if isinstance(inputs, dict)
                       else _inputs_f32(*inputs))
    runners = {rep: _runner(repeat=rep) for rep in (r_lo, r_hi)}
    dev_in = {rep: runners[rep].put_inputs(in_maps) for rep in (r_lo, r_hi)}
    for rep in (r_lo, r_hi):
        runners[rep].call(dev_in[rep])  # warm
    walls = {r_lo: [], r_hi: []}
    for _ in range(iters):
        for rep in (r_lo, r_hi):
            t0 = time.perf_counter()
            runners[rep].call(dev_in[rep])
            walls[rep].append(time.perf_counter() - t0)
    lo, hi = min(walls[r_lo]), min(walls[r_hi])
    ns = (hi - lo) * 1e9 / (r_hi - r_lo)
    return max(int(ns), 1), walls


# revision 25
# speedup vs baseline: 86.9161x; 1.1884x over previous
"""AttentionLSTM Trainium2 kernel — 8-core data-parallel.

Model (per batch row b): two independent single-direction LSTMs over T=43
steps of x[:, :, t] (H=300 features), hidden states summed, then a
conv-softmax attention over time, tanh, fc(300->80), softmax.

Device mapping per core (512 batch rows):
  - z^T[1200, 512] per (direction, step) via PE matmuls: merged contraction
    K=5 k-tiles of 128 (h rows 0..299 first, then bias + x tail, then x),
    M gate-aligned tiles {128,128,44}, fp16 MM inputs, fp32 PSUM accum.
  - h k-tiles are parity double-buffered: step t reads kt[t%2], writes h_t
    into kt[(t+1)%2], so every matmul of a step sees the full h_{t-1}
    (exact LSTM semantics, no Gauss-Seidel staleness).
  - group order is d-major (all of direction 0, then direction 1): d0's
    elementwise tail hides under d1's matmuls and vice versa across the
    step boundary, keeping the PE dense.
  - gates: one fused sigmoid over an [*,1024] i|f PSUM pair, tanh(g) and
    sigmoid(o) over a shared g|o PSUM pair; gate/cell elementwise state in
    fp16 for 2x DVE throughput; c stays in SBUF.
  - attention accumulated online: e_t = sigmoid(a)/(1-sigmoid(a)) = exp(a)
    (avoids exp table loads mid-loop); e_t broadcast across partitions with
    a rank-1 PE matmul (ones x e) into PSUM — no DRAM round trip; r += on
    GPSIMD.
  - tail: hStar = tanh(r/s), logits = fc(hStar) via PE (batch on PSUM
    partitions), softmax over the 80-class free dim.
"""

import os
import sys

sys.path.insert(0, "/opt/trn_rl_repo")

from contextlib import ExitStack

import numpy as np

import concourse.bass as bass
import concourse.tile as tile
from concourse import mybir
from concourse.bass_utils import run_bass_kernel_spmd  # noqa: F401  (spmd path kept available)

f32 = mybir.dt.float32
AF = mybir.ActivationFunctionType
AX = mybir.AxisListType

_BIRFIX_DONE = False


def _split_multiwaits(bir_json):
    """This walrus build allows one sync-wait per engine instruction; Tile
    attaches one per producer proc. Hoist extras onto standalone
    EventSemaphore instructions inserted just before, same engine queue."""
    import json
    j = json.loads(bir_json.decode() if isinstance(bir_json, bytes) else bir_json)
    for fn in j.get("functions", []):
        for blk in fn.get("blocks", []):
            out = []
            for ins in blk.get("instructions", []):
                si = ins.get("sync_info")
                ow = si.get("on_wait") if si else None
                if ow and len(ow) > 1:
                    for i, w in enumerate(ow[:-1]):
                        out.append({
                            "debug": ins.get("debug", 0),
                            "engine": ins["engine"],
                            "ins": [], "outs": [],
                            "name": f"{ins['name']}_xw{i}",
                            "opcode": "EventSemaphore",
                            "sync_info": {"on_update": [], "on_wait": [w]},
                        })
                    si["on_wait"] = [ow[-1]]
                out.append(ins)
            blk["instructions"] = out
    return json.dumps(j).encode()


def _install_birfix():
    global _BIRFIX_DONE
    if _BIRFIX_DONE:
        return
    from concourse import bass2jax
    orig = bass2jax.compile_bir_kernel

    def patched(bir_json, tmpdir, neff_name="file.neff"):
        return orig(_split_multiwaits(bir_json), tmpdir, neff_name)

    bass2jax.compile_bir_kernel = patched
    _BIRFIX_DONE = True


class _Runner:
    """Compile once; keep the sharded jitted executable + device inputs."""

    def __init__(self, nc, n_cores):
        import jax
        from jax.sharding import Mesh, PartitionSpec
        from jax.experimental.shard_map import shard_map
        from concourse import bass2jax as b2j

        b2j.install_neuronx_cc_hook()
        _install_birfix()
        self.jax = jax
        self.nc = nc
        self.n_cores = n_cores
        part_name = nc.partition_id_tensor.name if nc.partition_id_tensor else None
        in_names, out_names, out_avals, zero_outs = [], [], [], []
        for alloc in nc.m.functions[0].allocations:
            if not isinstance(alloc, mybir.MemoryLocationSet):
                continue
            name = alloc.memorylocations[0].name
            if alloc.kind == "ExternalInput":
                if name != part_name:
                    in_names.append(name)
            elif alloc.kind == "ExternalOutput":
                out_names.append(name)
                shape = tuple(alloc.tensor_shape)
                dtype = mybir.dt.np(alloc.dtype)
                out_avals.append(jax.core.ShapedArray(shape, dtype))
                zero_outs.append(np.zeros(shape, dtype))
        self.in_names = list(in_names)
        self.out_names = out_names
        self.out_avals = out_avals
        self.zero_outs = zero_outs
        n_params = len(in_names)
        n_outs = len(out_avals)
        all_names = in_names + out_names
        if part_name is not None:
            all_names = all_names + [part_name]
        donate = tuple(range(n_params, n_params + n_outs))

        def _body(*args):
            operands = list(args)
            if part_name is not None:
                operands.append(b2j.partition_id_tensor())
            outs = b2j._bass_exec_p.bind(
                *operands,
                out_avals=tuple(out_avals),
                in_names=tuple(all_names),
                out_names=tuple(out_names),
                lowering_input_output_aliases=(),
                sim_require_finite=True,
                sim_require_nnan=True,
                nc=nc,
            )
            return tuple(outs)

        devices = jax.devices()[:n_cores]
        self.mesh = Mesh(np.asarray(devices), ("core",))
        in_specs = (PartitionSpec("core"),) * (n_params + n_outs)
        out_specs = (PartitionSpec("core"),) * n_outs
        self.sharded = jax.jit(
            shard_map(_body, mesh=self.mesh, in_specs=in_specs,
                      out_specs=out_specs, check_rep=False),
            donate_argnums=donate, keep_unused=True)
        self.sharding = jax.sharding.NamedSharding(
            self.mesh, PartitionSpec("core"))

    def put_inputs(self, in_maps):
        jax = self.jax
        concat = [np.concatenate([np.asarray(m[n]) for m in in_maps], axis=0)
                  for n in self.in_names]
        return [jax.device_put(a, self.sharding) for a in concat]

    def call(self, dev_in):
        zeros = [np.zeros((self.n_cores * z.shape[0], *z.shape[1:]), z.dtype)
                 for z in self.zero_outs]
        outs = self.sharded(*dev_in, *zeros)
        self.jax.block_until_ready(outs)
        return outs

    def run(self, in_maps):
        dev_in = self.put_inputs(in_maps)
        outs = self.call(dev_in)
        n = self.n_cores
        return [
            {name: np.asarray(outs[i]).reshape(n, *self.out_avals[i].shape)[c]
             for i, name in enumerate(self.out_names)}
            for c in range(n)
        ]

    def bench(self, in_maps, iters=5):
        import time
        dev_in = self.put_inputs(in_maps)
        self.call(dev_in)  # warm
        times = []
        for _ in range(iters):
            t0 = time.perf_counter()
            self.call(dev_in)
            times.append(time.perf_counter() - t0)
        return times


B, H, T, NCLS = 4096, 300, 43, 80
NCORES = 8
BS = B // NCORES          # 512 batch rows per core
NK = 5                    # k-tiles: [h0:128 | h128:256 | h256:300+bias+xtail | x0:128 | x128:256]
BIASROW = 44              # partition of the bias (constant-1) row in k-tile 2
XTAIL = 64                # x rows 256..300 live at parts 64..108 of k-tile 2
MT = [(0, 128), (128, 128), (256, 44)]    # (moff, msz) per gate, output base partition 0
GOFF = [0, 300, 600, 900]                 # torch gate order i,f,g,o
WDRW = 4 * 304                            # fp8 DR weights: 16B-aligned gate pitch

MM_DT_NAME = os.environ.get("LSTM_MM_DT", "float16")
# fp8e4m3 DoubleRow for the x[0:256] contraction: one 256-row matmul pass
# replaces two fp16 128-row passes (20% fewer gate matmul passes; emulated
# end-to-end rel err 8.2e-3 vs the 2e-2 gate).
USE_DR = os.environ.get("LSTM_X_FP8_DR", "1") == "1"

_CACHE = {}


def _build(mdt_name, repeat=0, variant="full"):
    # variant: "full" | "no_attn" (skip attention accumulation) |
    # "no_dve" (also skip the c/h elementwise chain) | "mm_only"
    # (matmuls + DMAs only) | "mm_nodma" (matmuls, static rhs) |
    # "mm_n256" (matmuls at N=256).  Non-"full" variants are timing probes.
    do_attn = variant == "full"
    do_dve = variant in ("full", "no_attn", "no_rec")
    do_act = variant not in ("mm_only", "mm_nodma", "mm_n256")
    do_xdma = variant != "mm_nodma"
    do_rec = variant != "no_rec"
    ncols = 256 if variant == "mm_n256" else 512
    mdt = getattr(mybir.dt, mdt_name)
    f8 = mybir.dt.float8e4
    DRMODE = mybir.MatmulPerfMode.DoubleRow
    nc = bass.Bass(target_bir_lowering=False)

    xt_d = nc.declare_dram_parameter("xt", [T, 3, 128, BS], mdt, isOutput=False)
    if USE_DR:
        xdr_d = nc.declare_dram_parameter("xdr", [T, 128, 2, BS], f8,
                                          isOutput=False)
        wdr_d = nc.declare_dram_parameter("wdr", [2, 128, 2, WDRW], f8,
                                          isOutput=False)
    wc_d = nc.declare_dram_parameter("wc", [2, NK, 128, 1200], mdt, isOutput=False)
    conv_d = nc.declare_dram_parameter("convp", [128, 3], mdt, isOutput=False)
    fcw_d = nc.declare_dram_parameter("fcw", [128, 3 * NCLS], mdt, isOutput=False)
    fcb_d = nc.declare_dram_parameter("fcb", [1, NCLS], mdt, isOutput=False)
    ones_d = nc.declare_dram_parameter("onesrow", [1, BS], mdt, isOutput=False)
    out_d = nc.declare_dram_parameter("out", [BS, NCLS], f32, isOutput=True)

    with tile.TileContext(nc) as tc, ExitStack() as ctx:
        P = lambda name, bufs, **kw: ctx.enter_context(
            tc.tile_pool(name=name, bufs=bufs, **kw))
        wpool = P("w", 1)
        xpool = P("x", 3)
        # One shared pool for all gate PSUM tiles: 3 x [128,1024] f32 =
        # 6 banks.  With separate zif(bufs=2)/zgo(bufs=1) pools the g|o
        # matmuls of each group waited on the previous group's o-act drain
        # with ~0 margin -> ~1us PE stall per group (~260us/forward).
        zp = P("z", 3, space="PSUM")
        # Attention score [1,512] and broadcast [128,512] share one slot
        # tag (strictly sequential within a step); 2 bufs = 2 banks.
        atp = P("at", 2, space="PSUM")
        sifp = P("sif", 3)
        sop = P("so", 3)
        gcp = P("gc", 1)
        p1p = P("p1", 3)
        tcp = P("tc", 3)
        hp = P("h", 1)
        hsp = P("hs", 2)
        thp = P("th", 2)
        rp = P("r", 1)
        smp = P("sm", 2)
        tmpp = P("tmp", 2)
        fin = P("fin", 2)

        # ---- weights / constants ----
        wc_sb = {}
        nk_sb = 3 if USE_DR else NK
        for d in range(2):
            for k in range(nk_sb):
                wt = wpool.tile([128, 1200], mdt, tag=f"wc_{d}_{k}")
                nc.sync.dma_start(out=wt, in_=wc_d.ap()[d, k])
                wc_sb[(d, k)] = wt
        wdr_sb = {}
        if USE_DR:
            for d in range(2):
                wt = wpool.tile([128, 2, WDRW], f8, tag=f"wdr_{d}")
                nc.sync.dma_start(out=wt, in_=wdr_d.ap()[d])
                wdr_sb[d] = wt
        conv_sb = wpool.tile([128, 3], mdt, tag="conv")
        nc.sync.dma_start(out=conv_sb, in_=conv_d.ap())
        fcw_sb = wpool.tile([128, 3 * NCLS], mdt, tag="fcw")
        nc.sync.dma_start(out=fcw_sb, in_=fcw_d.ap())
        fcb_sb = wpool.tile([1, NCLS], mdt, tag="fcb")
        nc.sync.dma_start(out=fcb_sb, in_=fcb_d.ap())
        ones_sb = wpool.tile([1, 128], mdt, tag="ones")
        nc.vector.memset(ones_sb, 1.0)

        # ---- persistent state ----
        # h k-tiles, parity double-buffered: step t reads kt[t%2][d],
        # writes h_t into kt[(t+1)%2][d].
        kt = {}
        for par in range(2):
            for d in range(2):
                kt[(par, d)] = []
                for j in range(3):
                    t_ = hp.tile([128, BS], mdt, tag=f"kt_{par}_{d}_{j}")
                    nc.vector.memset(t_, 0.0)
                    kt[(par, d)].append(t_)
                nc.sync.dma_start(out=kt[(par, d)][2][BIASROW:BIASROW + 1],
                                  in_=ones_d.ap())
        gc = {}    # gc[(d, j)]: [128, 1024] mdt = [tanh_g | c]
        for d in range(2):
            for j in range(3):
                g = gcp.tile([128, 1024], mdt, tag=f"gc_{d}_{j}")
                nc.vector.memset(g, 0.0)
                gc[(d, j)] = g
        r = []
        for j in range(3):
            rt = rp.tile([128, BS], f32, tag=f"r_{j}")
            nc.vector.memset(rt, 0.0)
            r.append(rt)
        ssum = rp.tile([1, BS], f32, tag="ssum")
        # timing variants skip attention: keep 1/ssum finite in the tail
        nc.vector.memset(ssum, 0.0 if do_attn else 1.0)

        def w_slice(d, k, col0, msz):
            return wc_sb[(d, k)][:, col0:col0 + msz]

        def attn_tanh(hs):
            # hs[j]: [128, BS] mdt hsum tiles from the PREVIOUS step.
            th = []
            for j in range(3):
                pmax = 45 if j == 2 else 128
                thj = thp.tile([128, BS], mdt, tag=f"th{j}")
                nc.scalar.activation(out=thj[0:pmax], in_=hs[j][0:pmax],
                                     func=AF.Tanh)
                th.append((thj, pmax))
            return th

        def attn_score(th):
            a_ps = atp.tile([1, BS], f32, tag="at")
            for k in range(3):
                thj, pmax = th[k]
                nc.tensor.matmul(a_ps, lhsT=conv_sb[0:pmax, k:k + 1],
                                 rhs=thj[0:pmax], start=(k == 0), stop=(k == 2))
            sg = smp.tile([1, BS], f32, tag="sg")
            nc.scalar.activation(out=sg, in_=a_ps, func=AF.Sigmoid)
            om = smp.tile([1, BS], f32, tag="om")
            nc.scalar.activation(out=om, in_=sg, func=AF.Copy, bias=1.0,
                                 scale=-1.0)
            nc.vector.reciprocal(out=om, in_=om)
            e = smp.tile([1, BS], f32, tag="e")
            nc.vector.tensor_mul(out=e, in0=sg, in1=om)   # e = exp(a)
            nc.vector.tensor_add(out=ssum, in0=ssum, in1=e)
            e16 = smp.tile([1, BS], mdt, tag="e16")
            nc.scalar.activation(out=e16, in_=e, func=AF.Copy)
            return e16

        def attn_accum(hs, e16):
            eb_ps = atp.tile([128, BS], f32, tag="at")
            nc.tensor.matmul(eb_ps, lhsT=ones_sb, rhs=e16, start=True, stop=True)
            for j in range(3):
                pmax = 45 if j == 2 else 128
                tmp = tmpp.tile([128, BS], f32, tag=f"tmp{j}")
                nc.vector.tensor_mul(out=tmp[0:pmax], in0=hs[j][0:pmax],
                                     in1=eb_ps[0:pmax])
                nc.gpsimd.tensor_add(out=r[j][0:pmax], in0=r[j][0:pmax],
                                     in1=tmp[0:pmax])

        def attn_tail(hs):
            attn_accum(hs, attn_score(attn_tanh(hs)))

        loop_cm = tc.For_i(0, repeat, 1) if repeat else None
        if loop_cm is not None:
            loop_cm.__enter__()

        pending_hs = None
        if not do_xdma:
            if USE_DR:
                xdr0 = xpool.tile([128, 2, BS], f8, tag="xdr")
                nc.vector.memset(xdr0, 0.0)
            else:
                xa0 = xpool.tile([128, BS], mdt, tag="xa")
                nc.vector.memset(xa0, 0.0)
                xb0 = xpool.tile([128, BS], mdt, tag="xb")
                nc.vector.memset(xb0, 0.0)

        # ---- time loop ----
        for t in range(T):
            par, nxt = t % 2, (t + 1) % 2
            xa = xb = xdr = None
            if do_xdma:
                if USE_DR:
                    xdr = xpool.tile([128, 2, BS], f8, tag="xdr")
                    nc.sync.dma_start(out=xdr, in_=xdr_d.ap()[t])
                else:
                    xa = xpool.tile([128, BS], mdt, tag="xa")
                    nc.sync.dma_start(out=xa, in_=xt_d.ap()[t, 0])
                    xb = xpool.tile([128, BS], mdt, tag="xb")
                    nc.sync.dma_start(out=xb, in_=xt_d.ap()[t, 1])
                for d in range(2):
                    nc.sync.dma_start(out=kt[(par, d)][2][XTAIL:XTAIL + 44],
                                      in_=xt_d.ap()[t, 2][XTAIL:XTAIL + 44])
            elif USE_DR:
                xdr = xdr0
            else:
                xa, xb = xa0, xb0
            # previous step's attention tanh: emitted first so the th acts
            # drain ahead of this step's gate acts in the ACT FIFO.
            pend_th = attn_tanh(pending_hs) if (do_attn and pending_hs) else None
            pend_e16 = None

            hs = []
            for d in range(2):
                rhsk = [kt[(par, d)][0], kt[(par, d)][1], kt[(par, d)][2]]
                if not USE_DR:
                    rhsk += [xa, xb]
                for j, (moff, msz) in enumerate(MT):
                    sl = slice(0, msz)
                    zif = zp.tile([128, 1024], f32, tag="z")
                    zgo = zp.tile([128, 1024], f32, tag="z")
                    for gi, zdst in ((0, zif[sl, 0:ncols]),
                                     (1, zif[sl, 512:512 + ncols]),
                                     (2, zgo[sl, 0:ncols]),
                                     (3, zgo[sl, 512:512 + ncols])):
                        col0 = GOFF[gi] + moff
                        for k in range(len(rhsk)):
                            nc.tensor.matmul(
                                zdst, lhsT=w_slice(d, k, col0, msz),
                                rhs=rhsk[k][:, 0:ncols],
                                start=(k == 0), stop=(not USE_DR
                                                      and k == NK - 1))
                        if USE_DR:
                            # x[0:256] contribution: one fp8 DoubleRow pass
                            dc0 = 304 * gi + moff   # 16B-aligned gate starts
                            nc.tensor.matmul(
                                zdst, lhsT=wdr_sb[d][:, :, dc0:dc0 + msz],
                                rhs=xdr[:, :, 0:ncols],
                                start=False, stop=True, perf_mode=DRMODE)
                    if not do_act:
                        continue
                    sif = sifp.tile([128, 1024], mdt, tag="sif")
                    nc.scalar.activation(out=sif[sl], in_=zif[sl],
                                         func=AF.Sigmoid)
                    gcj = gc[(d, j)]
                    nc.scalar.activation(out=gcj[sl, 0:512], in_=zgo[sl, 0:512],
                                         func=AF.Tanh)
                    so = sop.tile([128, BS], mdt, tag="so")
                    nc.scalar.activation(out=so[sl], in_=zgo[sl, 512:1024],
                                         func=AF.Sigmoid)
                    if not do_dve:
                        continue
                    # c_new = sig_f * c + sig_i * tanh_g ; h = sig_o * tanh(c)
                    p1 = p1p.tile([128, 1024], mdt, tag="p1")
                    nc.vector.tensor_mul(out=p1[sl], in0=sif[sl], in1=gcj[sl])
                    nc.vector.tensor_add(out=gcj[sl, 512:1024],
                                         in0=p1[sl, 0:512], in1=p1[sl, 512:1024])
                    tcj = tcp.tile([128, BS], mdt, tag="tc")
                    nc.scalar.activation(out=tcj[sl], in_=gcj[sl, 512:1024],
                                         func=AF.Tanh)
                    # h_t lands directly in the next step's rhs k-tile
                    if do_rec:
                        hdst = kt[(nxt, d)][j]
                    else:  # timing probe: same traffic, no recurrence dep
                        hdst = tcp.tile([128, BS], mdt, tag="hscr")
                    nc.vector.tensor_mul(out=hdst[sl], in0=so[sl],
                                         in1=tcj[sl])
                    if d == 1 and do_attn:
                        pmax = 45 if j == 2 else 128
                        hsj = hsp.tile([128, BS], mdt, tag=f"hs{j}")
                        nc.vector.tensor_add(out=hsj[0:pmax],
                                             in0=kt[(nxt, 0)][j][0:pmax],
                                             in1=kt[(nxt, 1)][j][0:pmax])
                        hs.append(hsj)
                    # previous step's attention, staged so its PE ops never
                    # wait on its ACT/DVE chain: score after d0-j0 (conv
                    # matmuls see ready th, e16 chain overlaps d0-j1/j2),
                    # accumulate after d1-j0 (eb matmul sees ready e16).
                    if pend_th is not None:
                        if d == 0 and j == 1:
                            pend_e16 = attn_score(pend_th)
                        elif d == 1 and j == 0:
                            attn_accum(pending_hs, pend_e16)
            pending_hs = hs

        if do_attn:
            attn_tail(pending_hs)

        if loop_cm is not None:
            loop_cm.__exit__(None, None, None)

        # ---- tail: hStar = tanh(r / s); logits; softmax ----
        rs = smp.tile([1, BS], f32, tag="rs")
        nc.vector.reciprocal(out=rs, in_=ssum)
        rs16 = smp.tile([1, BS], mdt, tag="rs16")
        nc.scalar.activation(out=rs16, in_=rs, func=AF.Copy)
        rsb = atp.tile([128, BS], f32, tag="at")
        nc.tensor.matmul(rsb, lhsT=ones_sb, rhs=rs16, start=True, stop=True)
        hst = []
        for j in range(3):
            hn = fin.tile([128, BS], f32, tag=f"hn{j}")
            nc.vector.tensor_mul(out=hn, in0=r[j], in1=rsb)
            hj = fin.tile([128, BS], mdt, tag=f"hst{j}")
            nc.scalar.activation(out=hj, in_=hn, func=AF.Tanh)
            hst.append(hj)
        for bt in range(BS // 128):
            fcp = atp.tile([128, NCLS], f32, tag="at")
            for j in range(3):
                nc.tensor.matmul(fcp, lhsT=hst[j][:, bt * 128:(bt + 1) * 128],
                                 rhs=fcw_sb[:, j * NCLS:(j + 1) * NCLS],
                                 start=(j == 0), stop=False)
            nc.tensor.matmul(fcp, lhsT=ones_sb, rhs=fcb_sb, start=False, stop=True)
            mx = fin.tile([128, 1], f32, tag="mx")
            nc.vector.reduce_max(out=mx, in_=fcp, axis=AX.X)
            nmx = fin.tile([128, 1], f32, tag="nmx")
            nc.vector.tensor_scalar_mul(out=nmx, in0=mx, scalar1=-1.0)
            ex = fin.tile([128, NCLS], f32, tag="ex")
            nc.scalar.activation(out=ex, in_=fcp, func=AF.Exp, bias=nmx)
            sm = fin.tile([128, 1], f32, tag="smm")
            nc.vector.reduce_sum(out=sm, in_=ex, axis=AX.X)
            nc.vector.reciprocal(out=sm, in_=sm)
            ot = fin.tile([128, NCLS], f32, tag="ot")
            nc.vector.tensor_scalar_mul(out=ot, in0=ex, scalar1=sm)
            nc.sync.dma_start(out=out_d.ap()[bt * 128:(bt + 1) * 128], in_=ot)

    return nc


def _prep(x, w_ih, w_hh, b_ih, b_hh, conv_w, fc_w, fc_b, np_mdt):
    """Host-side layout prep (shared across cores + per-core x shards).

    Merged contraction rows (640 = 5 k-tiles of 128):
      tile 0: h[0:128]        tile 1: h[128:256]
      tile 2: h[256:300] at parts 0..43, bias (const-1 row) at part 44,
              x[256:300] at parts 64..107, zeros elsewhere
      tile 3: x[0:128]        tile 4: x[128:256]
    """
    bias = (b_ih + b_hh).astype(np.float32)  # [2, 1200]
    wc = np.zeros((2, NK, 128, 1200), np.float32)
    for d in range(2):
        comb = np.zeros((NK * 128, 1200), np.float32)
        comb[0:256] = w_hh[d].T[0:256]
        comb[256:300] = w_hh[d].T[256:300]
        comb[256 + BIASROW] = bias[d]
        comb[256 + XTAIL:256 + XTAIL + 44] = w_ih[d].T[256:300]
        comb[384:512] = w_ih[d].T[0:128]
        comb[512:640] = w_ih[d].T[128:256]
        wc[d] = comb.reshape(NK, 128, 1200)

    def h_pack(vec_or_mat, width):
        """Pack [300(, width)] h-feature data into the 3-tile h k-layout."""
        out = np.zeros((3, 128, width), np.float32)
        v = vec_or_mat.reshape(H, width)
        out[0] = v[0:128]
        out[1] = v[128:256]
        out[2, 0:44] = v[256:300]
        return out

    convp = np.ascontiguousarray(
        h_pack(conv_w, 1).reshape(3, 128).T)          # [128, 3]
    fcw = np.ascontiguousarray(
        h_pack(fc_w.T, NCLS).transpose(1, 0, 2).reshape(128, 3 * NCLS))

    shared = {
        "wc": wc.astype(np_mdt),
        "convp": convp.astype(np_mdt),
        "fcw": fcw.astype(np_mdt),
        "fcb": fc_b.reshape(1, NCLS).astype(np_mdt),
        "onesrow": np.ones((1, BS), np.float32).astype(np_mdt),
    }
    if USE_DR:
        import ml_dtypes
        np_f8 = ml_dtypes.float8_e4m3
        # wdr[d, p, s, 304*g + r] = w_ih[d][300*g + r, 128*s + p]
        wdr = np.zeros((2, 128, 2, WDRW), np.float32)
        for d in range(2):
            tmp = w_ih[d][:, 0:256].reshape(1200, 2, 128)
            for g in range(4):
                wdr[d, :, :, 304 * g:304 * g + 300] = (
                    tmp[300 * g:300 * g + 300].transpose(2, 1, 0))
        shared["wdr"] = wdr.astype(np_f8)

    # x: [B, H, T] -> per-core [T, 3, 128, BS]:
    # slot 0 = x[0:128], slot 1 = x[128:256],
    # slot 2 = zeros with x[256:300] at parts 64..107.
    xs = np.ascontiguousarray(np.transpose(x, (2, 1, 0)))  # [T, H, B]
    xp = np.zeros((T, 3, 128, B), np.float32)
    xp[:, 0] = xs[:, 0:128]
    xp[:, 1] = xs[:, 128:256]
    xp[:, 2, XTAIL:XTAIL + 44] = xs[:, 256:300]
    xp = xp.reshape(T, 3, 128, NCORES, BS)
    if USE_DR:
        # xdr[t, p, s, b] = x[t, 128*s + p, b]
        xdr = xs[:, 0:256].reshape(T, 2, 128, NCORES, BS).transpose(0, 2, 1, 3, 4)
    in_maps = []
    for c in range(NCORES):
        m = dict(shared)
        m["xt"] = np.ascontiguousarray(xp[:, :, :, c]).astype(np_mdt)
        if USE_DR:
            import ml_dtypes
            m["xdr"] = np.ascontiguousarray(xdr[:, :, :, c]).astype(
                ml_dtypes.float8_e4m3)
        in_maps.append(m)
    return in_maps


def _np_mdt(mdt_name):
    return np.float16 if mdt_name == "float16" else (
        __import__("ml_dtypes").bfloat16 if mdt_name == "bfloat16" else np.float32)


def _runner(repeat=0, variant="full"):
    key = (MM_DT_NAME, repeat, variant)
    if key not in _CACHE:
        _CACHE[key] = _Runner(_build(MM_DT_NAME, repeat=repeat,
                                     variant=variant), NCORES)
    return _CACHE[key]


def _in_maps(inputs_f32):
    return _prep(*inputs_f32, _np_mdt(MM_DT_NAME))


def _inputs_f32(x, w_ih, w_hh, b_ih, b_hh, conv_w, fc_w, fc_b):
    return [np.asarray(a, np.float32) for a in
            (x, w_ih, w_hh, b_ih, b_hh, conv_w, fc_w, fc_b)]


def kernel(x, w_ih, w_hh, b_ih, b_hh, conv_w, fc_w, fc_b):
    runner = _runner(repeat=0)
    in_maps = _in_maps(_inputs_f32(x, w_ih, w_hh, b_ih, b_hh,
                                   conv_w, fc_w, fc_b))
    results = runner.run(in_maps)
    out = np.concatenate([r["out"] for r in results], axis=0)
    return out.astype(np.float32)


def bench(x, w_ih, w_hh, b_ih, b_hh, conv_w, fc_w, fc_b, iters=5):
    runner = _runner(repeat=0)
    in_maps = _in_maps(_inputs_f32(x, w_ih, w_hh, b_ih, b_hh,
                                   conv_w, fc_w, fc_b))
    return runner.bench(in_maps, iters=iters)


def measure_exec_ns(inputs, r_lo=1, r_hi=301, iters=10):
    """Device execution time of one full forward pass, in ns.

    The axon tunnel adds a fixed ~70-80 ms completion-notification latency
    to every blocking call, independent of what the NEFF does (measured:
    a trivial 4-instruction kernel takes the same wall time as the full
    LSTM).  To measure hardware execution, both builds wrap the whole
    T-step forward in a hardware For_i loop (r_lo vs r_hi iterations,
    identical instruction stream per iteration); the slope
    (min_wall(r_hi) - min_wall(r_lo)) / (r_hi - r_lo) is the steady-state
    on-device time of one forward pass with the constant latency cancelled.
    Samples are interleaved so network drift affects both arms equally.
    """
    import time
    in_maps = _in_maps(_inputs_f32(**inputs) if isinstance(inputs, dict)
                       else _inputs_f32(*inputs))
    runners = {rep: _runner(repeat=rep) for rep in (r_lo, r_hi)}
    dev_in = {rep: runners[rep].put_inputs(in_maps) for rep in (r_lo, r_hi)}
    for rep in (r_lo, r_hi):
        runners[rep].call(dev_in[rep])  # warm
    walls = {r_lo: [], r_hi: []}
    for _ in range(iters):
        for rep in (r_lo, r_hi):
            t0 = time.perf_counter()
            runners[rep].call(dev_in[rep])
            walls[rep].append(time.perf_counter() - t0)
    lo, hi = min(walls[r_lo]), min(walls[r_hi])
    ns = (hi - lo) * 1e9 / (r_hi - r_lo)
    return max(int(ns), 1), walls
